# revision 1
# baseline (speedup 1.0000x reference)
"""ByteContextEncoder Trainium2 kernel.

8-core SPMD sharding: core c handles batch row c//2, sequence half c%2
(T_loc = 1024 tokens). Attention needs full-row K/V -> pair AllGather
([0,1],[2,3],[4,5],[6,7]). Segment mean pooling is done with one-hot
gather/scatter matmuls; the one segment that can span the half boundary is
fixed up with a tiny pair AllGather of its partial sum.

All float tensor compute runs on device in bf16 (f32 residual/psum).
Host only builds index structures (one-hot matrices, rope tables, counts)
derived from the integer `tokens` input, and casts weights to bf16.
"""

import math

import numpy as np
import ml_dtypes

import concourse.bass as bass
import concourse.mybir as mybir
import concourse.tile as tile
from concourse.masks import make_identity

BF16 = mybir.dt.bfloat16
F32 = mybir.dt.float32
AX = mybir.AxisListType
ALU = mybir.AluOpType
ACT = mybir.ActivationFunctionType

# model dims (hardcoded per problem spec)
B, T, D, H, L = 4, 2048, 512, 8, 2
FF = 4 * D
HD = D // H
EPS = 1e-6
ALPHA = 0.5

N_CORES = 8

# tunables (cost-model sweep)
CFG = {
    "mm_bufs": 2,
    "st_bufs": 2,
    "tr_bufs": 2,
    "tmp_bufs": 4,
    "stp_bufs": 2,
    "wsm_bufs": 3,
    "wffc_bufs": 4,
}
TL = T // 2          # tokens per core
P = 128
NT = TL // P         # 8 local token tiles
KT2 = T // P         # 16 full-row token tiles
DC = D // P          # 4 D chunks
FFC = FF // P        # 16 FF chunks
SEG = 384            # padded segments per core
SC = SEG // P        # 3 segment chunks
QBW = 256            # q-block width for attention
NQB = TL // QBW      # 4 q blocks

_SEP = b" \t\n\r.,;:!?()[]{}\"'" + b"+-*/=<>|&^~%@#$\\"
SEP_TABLE = np.zeros(256, dtype=bool)
SEP_TABLE[list(_SEP)] = True

KV_K_BYTES = D * TL            # kT elements (bf16 count)
KV_V_BYTES = TL * H * (HD + 1)  # v+ones elements
KV_TOT = KV_K_BYTES + KV_V_BYTES


def split_multiwait_drains(nc, max_waits=1):
    """This container's walrus can't encode >1 sync-wait on an instruction;
    hoist extra waits onto single-wait NoOps just before it (same engine, so
    sequencer order preserves the wait-before-execute semantics)."""
    n_patched = 0
    for f in nc.m.functions:
        for bb in f.blocks:
            new_list = []
            changed = False
            for ins in bb.instructions:
                si = ins.sync_info
                if si is not None and si.on_wait and len(si.on_wait) > max_waits:
                    for k, w in enumerate(si.on_wait):
                        nop = mybir.InstNoOp(name=f"{ins.name}-w{k}", ins=[], outs=[])
                        nop.engine = ins.engine
                        nop.sync_info = mybir.SyncInfo(on_wait=[w], on_update=[])
                        new_list.append(nop)
                    ins.sync_info = mybir.SyncInfo(
                        on_wait=[], on_update=list(si.on_update)
                    )
                    changed = True
                    n_patched += 1
                new_list.append(ins)
            if changed:
                bb.instructions = new_list
    return n_patched


def build_program(debug=(), patch=True, stage=5, for_sim=False, apply_fw=False):
    nc = bass.Bass(num_devices=N_CORES)

    # ---------------- DRAM inputs ----------------
    d_emb = nc.dram_tensor("emb", [256, D], BF16, kind="ExternalInput")
    d_ident = nc.dram_tensor("ident", [P, P], BF16, kind="ExternalInput")
    d_obind = nc.dram_tensor("obind", [P, TL], BF16, kind="ExternalInput")
    d_oet = nc.dram_tensor("oet", [256, TL], BF16, kind="ExternalInput")
    d_og = nc.dram_tensor("og", [TL, SEG], BF16, kind="ExternalInput")
    d_otg = nc.dram_tensor("otg", [SEG, TL], BF16, kind="ExternalInput")
    d_icnt = nc.dram_tensor("icnt", [SEG], F32, kind="ExternalInput")
    d_wfx = nc.dram_tensor("wfx", [P, SEG], F32, kind="ExternalInput")
    d_esnd = nc.dram_tensor("esnd", [SEG, P], BF16, kind="ExternalInput")
    d_fw = nc.dram_tensor("fw", [D], F32, kind="ExternalInput")
    d_cos = nc.dram_tensor("cos", [P, TL], BF16, kind="ExternalInput")
    d_sin = nc.dram_tensor("sin", [P, TL], BF16, kind="ExternalInput")
    d_cosf = nc.dram_tensor("cosf", [P, T], BF16, kind="ExternalInput")
    d_sinf = nc.dram_tensor("sinf", [P, T], BF16, kind="ExternalInput")
    d_wq = nc.dram_tensor("wq", [L, D, D], BF16, kind="ExternalInput")
    d_wqr = nc.dram_tensor("wqr", [L, D, D], BF16, kind="ExternalInput")
    d_wk = nc.dram_tensor("wk", [L, D, D], BF16, kind="ExternalInput")
    d_wkr = nc.dram_tensor("wkr", [L, D, D], BF16, kind="ExternalInput")
    d_wv = nc.dram_tensor("wv", [L, D, D], BF16, kind="ExternalInput")
    d_wo = nc.dram_tensor("wo", [L, D, D], BF16, kind="ExternalInput")
    d_w1 = nc.dram_tensor("w1", [L, D, FF], BF16, kind="ExternalInput")
    d_w2 = nc.dram_tensor("w2", [L, D, FF], BF16, kind="ExternalInput")
    d_w3 = nc.dram_tensor("w3", [L, FF, D], BF16, kind="ExternalInput")

    d_y = nc.dram_tensor("y", [TL, D], F32, kind="ExternalOutput")
    dbg_out = {}

    def dbg(name, shape, dtype=F32):
        if name in debug:
            dbg_out[name] = nc.dram_tensor(
                "dbg_" + name, shape, dtype, kind="ExternalOutput"
            )
            return dbg_out[name]
        return None

    with tile.TileContext(nc) as tc:
        with (
            tc.tile_pool(name="state", bufs=1) as state,
            tc.tile_pool(name="aux", bufs=1) as aux,
            tc.tile_pool(name="wsm", bufs=CFG["wsm_bufs"]) as wsm,
            tc.tile_pool(name="wff", bufs=CFG["wffc_bufs"]) as wff,
            tc.tile_pool(name="w3p", bufs=1) as w3p,
            tc.tile_pool(name="stp", bufs=CFG["stp_bufs"]) as stp,
            tc.tile_pool(name="tmp", bufs=CFG["tmp_bufs"]) as tmp,
            tc.tile_pool(name="psum", bufs=CFG["mm_bufs"], space="PSUM") as psum,
            tc.tile_pool(name="psum_st", bufs=CFG["st_bufs"], space="PSUM") as psum_st,
            tc.tile_pool(name="psum_tr", bufs=CFG["tr_bufs"], space="PSUM") as psum_tr,
            tc.tile_pool(name="dram", bufs=1, space="DRAM") as dram,
        ):
            # ---- persistent state ----
            x_sb = state.tile([P, NT, D], F32, tag="x")          # residual
            cos_sb = state.tile([P, TL], BF16, tag="cos")
            sin_sb = state.tile([P, TL], BF16, tag="sin")
            cosf_sb = state.tile([P, T], BF16, tag="cosf")
            sinf_sb = state.tile([P, T], BF16, tag="sinf")
            ident = state.tile([P, P], BF16, tag="ident")
            eps_sb = state.tile([P, 1], F32, tag="eps")
            nc.vector.memset(eps_sb[:], EPS)

            # ---- embedding: x = onehot @ table (inputs loaded first) ----
            embt = aux.tile([P, 2, D], BF16, tag="otg_embt")
            oet = aux.tile([P, 2, TL], BF16, tag="og_oet")
            nc.sync.dma_start(oet[:], d_oet.rearrange("(c p) t -> p c t", p=P))
            nc.sync.dma_start(embt[:], d_emb.rearrange("(c p) d -> p c d", p=P))
            nc.sync.dma_start(ident[:], d_ident[:])
            nc.sync.dma_start(cos_sb[:], d_cos[:])
            nc.sync.dma_start(sin_sb[:], d_sin[:])
            nc.sync.dma_start(cosf_sb[:], d_cosf[:])
            nc.sync.dma_start(sinf_sb[:], d_sinf[:])
            for t in range(NT):
                ps = psum.tile([P, 512], F32, tag="mm")
                for kc in range(2):
                    nc.tensor.matmul(
                        ps[:],
                        oet[:, kc, t * P : (t + 1) * P],
                        embt[:, kc, :],
                        start=(kc == 0),
                        stop=(kc == 1),
                    )
                nc.scalar.copy(x_sb[:, t, :], ps[:])

            def rmsnorm_tile(t, out_tile, out_slice):
                """out = x_sb[:,t,:] * rsqrt(mean(x^2)+eps), bf16."""
                xsq = tmp.tile([P, D], BF16, tag="h")
                ssq = tmp.tile([P, 1], F32, tag="ssq")
                nc.scalar.activation(
                    xsq[:], x_sb[:, t, :], ACT.Square, accum_out=ssq[:]
                )
                nc.scalar.activation(
                    ssq[:], ssq[:], ACT.Sqrt, bias=eps_sb[:], scale=1.0 / D
                )
                nc.vector.reciprocal(ssq[:], ssq[:])
                nc.vector.tensor_scalar_mul(out_tile[out_slice], x_sb[:, t, :], ssq[:])

            def transpose_into(dst, dst_c, dst_t, src):
                """src (128 tok,512) bf16 -> dst[:, c, t*128:...] for c in 0..3"""
                for c in range(DC):
                    pt = psum_tr.tile([P, P], BF16, tag="tr")
                    nc.tensor.transpose(pt[:], src[:, c * P : (c + 1) * P], ident[:])
                    nc.scalar.copy(dst[:, dst_c + c, dst_t * P : (dst_t + 1) * P], pt[:])

            # pooling index matrices: trace early so their DMAs overlap layers
            og_e = aux.tile([P, NT, SEG], BF16, tag="og_oet")
            otg_e = aux.tile([P, SC, TL], BF16, tag="otg_embt")
            icnt_e = aux.tile([P, SC, 1], F32, tag="icnt")
            esnd_e = aux.tile([P, SC, P], BF16, tag="esnd")
            obind_e = aux.tile([P, TL], BF16, tag="obind")
            pool_aux = (og_e, otg_e, icnt_e, esnd_e, obind_e)
            nc.sync.dma_start(og_e[:], d_og.rearrange("(c p) s -> p c s", p=P))
            nc.sync.dma_start(otg_e[:], d_otg.rearrange("(c p) t -> p c t", p=P))
            nc.sync.dma_start(
                icnt_e[:], d_icnt.rearrange("(c p) -> p c", p=P)[:, :, None]
            )
            nc.sync.dma_start(esnd_e[:], d_esnd.rearrange("(c p) m -> p c m", p=P))
            nc.sync.dma_start(obind_e[:], d_obind[:])

            # ================= layers =================
            for l in range(L if stage >= 4 else (1 if stage >= 2 else 0)):
                hT = state.tile([P, DC, TL], BF16, tag="oT")
                for t in range(NT):
                    h_t = tmp.tile([P, D], BF16, tag="h")
                    rmsnorm_tile(t, h_t, np.s_[:])
                    transpose_into(hT, 0, t, h_t)

                # ---- exchange h within pair (full-row h needed for K/V) ----
                bh_in = dram.tile([D * TL], BF16, tag="bkv_in")
                bh_out = dram.tile([2 * D * TL], BF16, tag="bkv_out")
                for mc in range(DC):
                    nc.sync.dma_start(
                        bh_in[mc * P * TL : (mc + 1) * P * TL].rearrange(
                            "(p x) -> p x", p=P
                        ),
                        hT[:, mc, :],
                    )
                nc.gpsimd.collective_compute(
                    "AllGather",
                    ALU.bypass,
                    replica_groups=[[0, 1], [2, 3], [4, 5], [6, 7]],
                    ins=[bh_in[:].opt()],
                    outs=[bh_out[:].opt()],
                )

                # ---- q projection + rope from local h (overlaps collective) ----
                qT = state.tile([P, DC, TL], BF16, tag="h12")
                w_sb = wsm.tile([P, DC, D], BF16, tag="wsm")
                wr_sb = wsm.tile([P, DC, D], BF16, tag="wsm")
                nc.sync.dma_start(w_sb[:], d_wq[l].rearrange("(c p) n -> p c n", p=P))
                nc.sync.dma_start(wr_sb[:], d_wqr[l].rearrange("(c p) n -> p c n", p=P))
                for mc in range(DC):
                    for nh in range(2):
                        nsl = np.s_[nh * 512 : (nh + 1) * 512]
                        pa = psum.tile([P, 512], F32, tag="mm")
                        pb0 = psum_st.tile([P, 4, QBW], F32, tag="st", name="pb0")
                        pb = pb0.rearrange("p a b -> p (a b)")[:, :512]
                        for kc in range(DC):
                            nc.tensor.matmul(
                                pa[:],
                                w_sb[:, kc, mc * P : (mc + 1) * P],
                                hT[:, kc, nsl],
                                start=(kc == 0),
                                stop=(kc == DC - 1),
                            )
                        for kc in range(DC):
                            nc.tensor.matmul(
                                pb[:],
                                wr_sb[:, kc, mc * P : (mc + 1) * P],
                                hT[:, kc, nsl],
                                start=(kc == 0),
                                stop=(kc == DC - 1),
                            )
                        t1 = tmp.tile([P, 512], BF16, tag="rope")
                        t2 = tmp.tile([P, 512], BF16, tag="rope")
                        nc.vector.tensor_mul(t1[:], pa[:], cos_sb[:, nsl])
                        nc.vector.tensor_mul(t2[:], pb[:], sin_sb[:, nsl])
                        nc.vector.tensor_add(qT[:, mc, nsl], t1[:], t2[:])

                # ---- gathered full-row h ----
                hTf = state.tile([P, DC, T], BF16, tag="hTf")
                for r in range(2):
                    for mc in range(DC):
                        nc.sync.dma_start(
                            hTf[:, mc, r * TL : (r + 1) * TL],
                            bh_out[r * D * TL + mc * P * TL :][: P * TL].rearrange(
                                "(p x) -> p x", p=P
                            ),
                        )

                # ---- k projection + rope, full row -> kTf ----
                # two tiles so heads 0-3 attention can start before heads 4-7
                # projections finish
                kTfA = state.tile([P, 2, T], BF16, tag="kTfA")
                kTfB = state.tile([P, 2, T], BF16, tag="kTfB")
                wk_sb = wsm.tile([P, DC, D], BF16, tag="wsm")
                wkr_sb = wsm.tile([P, DC, D], BF16, tag="wsm")
                nc.sync.dma_start(wk_sb[:], d_wk[l].rearrange("(c p) n -> p c n", p=P))
                nc.sync.dma_start(
                    wkr_sb[:], d_wkr[l].rearrange("(c p) n -> p c n", p=P)
                )
                for mc in range(DC):
                    for nh in range(4):
                        nsl = np.s_[nh * 512 : (nh + 1) * 512]
                        pa = psum.tile([P, 512], F32, tag="mm")
                        pb0 = psum_st.tile([P, 4, QBW], F32, tag="st", name="pb0")
                        pb = pb0.rearrange("p a b -> p (a b)")[:, :512]
                        for kc in range(DC):
                            nc.tensor.matmul(
                                pa[:],
                                wk_sb[:, kc, mc * P : (mc + 1) * P],
                                hTf[:, kc, nsl],
                                start=(kc == 0),
                                stop=(kc == DC - 1),
                            )
                        for kc in range(DC):
                            nc.tensor.matmul(
                                pb[:],
                                wkr_sb[:, kc, mc * P : (mc + 1) * P],
                                hTf[:, kc, nsl],
                                start=(kc == 0),
                                stop=(kc == DC - 1),
                            )
                        t1 = tmp.tile([P, 512], BF16, tag="rope")
                        t2 = tmp.tile([P, 512], BF16, tag="rope")
                        nc.vector.tensor_mul(t1[:], pa[:], cosf_sb[:, nsl])
                        nc.vector.tensor_mul(t2[:], pb[:], sinf_sb[:, nsl])
                        ktile = kTfA if mc < 2 else kTfB
                        nc.vector.tensor_add(ktile[:, mc % 2, nsl], t1[:], t2[:])

                # ---- v (tokens layout + ones col), full row -> vf ----
                vf = state.tile([P, KT2, H, HD + 1], BF16, tag="vf")
                wv_sb = wsm.tile([P, DC, D], BF16, tag="wsm")
                nc.sync.dma_start(wv_sb[:], d_wv[l].rearrange("(c p) n -> p c n", p=P))
                for t in range(KT2):
                    ps = psum.tile([P, 512], F32, tag="mm")
                    for kc in range(DC):
                        nc.tensor.matmul(
                            ps[:],
                            hTf[:, kc, t * P : (t + 1) * P],
                            wv_sb[:, kc, :],
                            start=(kc == 0),
                            stop=(kc == DC - 1),
                        )
                    nc.vector.tensor_copy(
                        vf[:, t, :, :HD], ps[:].rearrange("p (h d) -> p h d", h=H)
                    )
                    nc.vector.memset(vf[:, t, :, HD], 1.0)

                # ---- attention ----
                if stage < 3:
                    continue
                o_sb = state.tile([P, NT, D], BF16, tag="o_or_xn")
                for h in range(H):
                    po = (h % 2) * HD
                    hc = h // 2
                    for qb in range(NQB):
                        qsl = np.s_[qb * QBW : (qb + 1) * QBW]
                        st_sb = stp.tile([P, KT2, QBW], BF16, tag="st")
                        for k4 in range(KT2 // 4):
                            pst = psum_st.tile([P, 4, QBW], F32, tag="st")
                            for j in range(4):
                                kt = k4 * 4 + j
                                ksrc = kTfA if hc < 2 else kTfB
                                nc.tensor.matmul(
                                    pst[:, j, :],
                                    ksrc[po : po + HD, hc % 2, kt * P : (kt + 1) * P],
                                    qT[po : po + HD, hc, qsl],
                                    start=True,
                                    stop=True,
                                )
                            nc.scalar.activation(
                                st_sb[:, k4 * 4 : (k4 + 1) * 4, :],
                                pst[:],
                                ACT.Exp,
                                scale=1.0 / math.sqrt(HD),
                            )
                        for qt in range(QBW // P):
                            tix = qb * (QBW // P) + qt
                            pav = psum.tile([P, 512], F32, tag="mm")
                            for kt in range(KT2):
                                nc.tensor.matmul(
                                    pav[:, : HD + 1],
                                    st_sb[:, kt, qt * P : (qt + 1) * P],
                                    vf[:, kt, h, :],
                                    start=(kt == 0),
                                    stop=(kt == KT2 - 1),
                                )
                            rcp = tmp.tile([P, 1], F32, tag="rcp")
                            nc.vector.reciprocal(rcp[:], pav[:, HD : HD + 1])
                            nc.vector.tensor_scalar_mul(
                                o_sb[:, tix, h * HD : (h + 1) * HD],
                                pav[:, :HD],
                                rcp[:],
                            )

                # o -> oT -> wo -> residual
                oT = state.tile([P, DC, TL], BF16, tag="oT")
                for t in range(NT):
                    transpose_into(oT, 0, t, o_sb[:, t, :])
                wo_sb = wsm.tile([P, DC, D], BF16, tag="wsm")
                nc.sync.dma_start(wo_sb[:], d_wo[l].rearrange("(c p) n -> p c n", p=P))
                for t in range(NT):
                    ps = psum.tile([P, 512], F32, tag="mm")
                    for kc in range(DC):
                        nc.tensor.matmul(
                            ps[:],
                            oT[:, kc, t * P : (t + 1) * P],
                            wo_sb[:, kc, :],
                            start=(kc == 0),
                            stop=(kc == DC - 1),
                        )
                    nc.vector.tensor_add(x_sb[:, t, :], x_sb[:, t, :], ps[:])

                # ---- FFN ----
                if stage < 4:
                    continue
                hT2 = state.tile([P, DC, TL], BF16, tag="oT")
                for t in range(NT):
                    h_t = tmp.tile([P, D], BF16, tag="h")
                    rmsnorm_tile(t, h_t, np.s_[:])
                    transpose_into(hT2, 0, t, h_t)

                w3_sb = w3p.tile([P, FFC, D], BF16, tag="w3")
                nc.sync.dma_start(w3_sb[:], d_w3[l].rearrange("(c p) n -> p c n", p=P))
                for th in range(2):
                    nsl = np.s_[th * 512 : (th + 1) * 512]
                    h12 = state.tile([P, FFC, 512], BF16, tag="h12")
                    for mc2 in range(FFC // 2):
                        msl = np.s_[:, mc2 * 2 * P : (mc2 + 1) * 2 * P]
                        w1c = wff.tile([P, DC, 2 * P], BF16, tag="wffc")
                        w2c = wff.tile([P, DC, 2 * P], BF16, tag="wffc")
                        nc.sync.dma_start(
                            w1c[:], d_w1[l][msl].rearrange("(c p) n -> p c n", p=P)
                        )
                        nc.sync.dma_start(
                            w2c[:], d_w2[l][msl].rearrange("(c p) n -> p c n", p=P)
                        )
                        for mi in range(2):
                            mc = mc2 * 2 + mi
                            p1 = psum.tile([P, 512], F32, tag="mm")
                            p20 = psum_st.tile([P, 4, QBW], F32, tag="st", name="p20")
                            p2 = p20.rearrange("p a b -> p (a b)")[:, :512]
                            for kc in range(DC):
                                nc.tensor.matmul(
                                    p1[:],
                                    w1c[:, kc, mi * P : (mi + 1) * P],
                                    hT2[:, kc, nsl],
                                    start=(kc == 0),
                                    stop=(kc == DC - 1),
                                )
                            for kc in range(DC):
                                nc.tensor.matmul(
                                    p2[:],
                                    w2c[:, kc, mi * P : (mi + 1) * P],
                                    hT2[:, kc, nsl],
                                    start=(kc == 0),
                                    stop=(kc == DC - 1),
                                )
                            sl = tmp.tile([P, 512], BF16, tag="h")
                            if for_sim:
                                nc.scalar.activation(sl[:], p1[:], ACT.Sigmoid)
                                u = tmp.tile([P, 512], BF16, tag="h")
                                nc.vector.tensor_mul(u[:], p1[:], sl[:])
                                nc.vector.tensor_mul(h12[:, mc, :], p2[:], u[:])
                            else:
                                nc.scalar.activation(sl[:], p1[:], ACT.Silu)
                                nc.vector.tensor_mul(h12[:, mc, :], p2[:], sl[:])

                    for t in range(th * 4, th * 4 + 4):
                        ps = psum.tile([P, 512], F32, tag="mm")
                        for kc in range(FFC):
                            nc.tensor.matmul(
                                ps[:],
                                h12[:, kc, (t - th * 4) * P : (t - th * 4 + 1) * P],
                                w3_sb[:, kc, :],
                                start=(kc == 0),
                                stop=(kc == FFC - 1),
                            )
                        nc.vector.tensor_add(x_sb[:, t, :], x_sb[:, t, :], ps[:])

            # ================= final norm + segment pooling =================
            xn = state.tile([P, NT, D], BF16, tag="o_or_xn")
            for t in range(NT):
                rmsnorm_tile(t, xn, np.s_[:, t, :])

            if stage < 5:
                for t in range(NT):
                    xf = tmp.tile([P, D], F32, tag="f32t")
                    nc.vector.tensor_copy(xf[:], xn[:, t, :])
                    nc.sync.dma_start(
                        d_y.rearrange("(t p) d -> t p d", p=P)[t], xf[:]
                    )
                og = None
            if stage >= 5:
                og, otg, icnt, esnd, obind = pool_aux
                if apply_fw:
                    fw_bc = aux.tile([P, D], F32, tag="fw_bc")
                    nc.sync.dma_start(
                        fw_bc[:],
                        bass.AP(tensor=d_fw, offset=0, ap=[[0, P], [1, D]]),
                    )

                segsum_bf = aux.tile([P, SC, D], BF16, tag="segsum_bf")
                for mc in range(SC):
                    ps = psum.tile([P, 512], F32, tag="mm")
                    for kt in range(NT):
                        nc.tensor.matmul(
                            ps[:],
                            og[:, kt, mc * P : (mc + 1) * P],
                            xn[:, kt, :],
                            start=(kt == 0),
                            stop=(kt == NT - 1),
                        )
                    nc.scalar.copy(segsum_bf[:, mc, :], ps[:])

                # extract my boundary partial, exchange within pair
                bseg_in = dram.tile([D], F32, tag="bseg_in")
                bseg_out = dram.tile([2, D], F32, tag="bseg_out")
                pex = psum.tile([P, 512], F32, tag="mm")
                for kc in range(SC):
                    nc.tensor.matmul(
                        pex[:],
                        esnd[:, kc, :],
                        segsum_bf[:, kc, :],
                        start=(kc == 0),
                        stop=(kc == SC - 1),
                    )
                bpart = tmp.tile([1, D], F32, tag="bx")
                nc.vector.tensor_copy(bpart[:], pex[:1, :])
                nc.sync.dma_start(bseg_in[:].rearrange("(a d) -> a d", a=1), bpart[:])
                nc.gpsimd.collective_compute(
                    "AllGather",
                    ALU.bypass,
                    replica_groups=[[0, 1], [2, 3], [4, 5], [6, 7]],
                    ins=[bseg_in[:].opt()],
                    outs=[bseg_out[:].opt()],
                )
                bg = tmp.tile([P, D], F32, tag="bx")
                nc.vector.memset(bg[:], 0.0)
                nc.sync.dma_start(bg[:2, :], bseg_out[:])

                # seg_mean (no boundary fix) = my_partial * icnt; the partner
                # contribution is added as a rank-1 correction matmul below so
                # the main scatter does not wait on the collective.
                segmean = aux.tile([P, SC, D], BF16, tag="segmean")
                for mc in range(SC):
                    nc.vector.tensor_scalar_mul(
                        segmean[:, mc, :], segsum_bf[:, mc, :], icnt[:, mc, :]
                    )
                bgfix = tmp.tile([P, D], BF16, tag="h")
                nc.vector.tensor_copy(bgfix[:], bg[:])

                # out = 0.5*xn + scatter(segmean) + obind.T @ partner_partial
                for t in range(NT):
                    ps = psum.tile([P, 512], F32, tag="mm")
                    for kc in range(SC):
                        nc.tensor.matmul(
                            ps[:],
                            otg[:, kc, t * P : (t + 1) * P],
                            segmean[:, kc, :],
                            start=(kc == 0),
                            stop=False,
                        )
                    nc.tensor.matmul(
                        ps[:],
                        obind[:, t * P : (t + 1) * P],
                        bgfix[:],
                        start=False,
                        stop=True,
                    )
                    xh = tmp.tile([P, D], F32, tag="f32t")
                    nc.vector.tensor_scalar_mul(xh[:], xn[:, t, :], ALPHA)
                    out_t = tmp.tile([P, D], F32, tag="f32t")
                    nc.vector.tensor_add(out_t[:], ps[:], xh[:])
                    if apply_fw:
                        nc.vector.tensor_mul(out_t[:], out_t[:], fw_bc[:])
                    nc.sync.dma_start(
                        d_y.rearrange("(t p) d -> t p d", p=P)[t], out_t[:]
                    )

            # ---- debug taps ----
            if "x0" in debug:
                nc.sync.dma_start(
                    dbg_out["x0"].rearrange("(t p) d -> t p d", p=P)[:], x_sb[:]
                )

    if patch:
        split_multiwait_drains(nc)
    return nc


# ----------------------------------------------------------------------------
# host side
# ----------------------------------------------------------------------------


def _rot_cols(w):
    """Columns permuted/negated so (h @ w_rot) == rotate_half(h @ w)."""
    wr = np.empty_like(w)
    for hb in range(0, D, HD):
        wr[:, hb : hb + HD // 2] = -w[:, hb + HD // 2 : hb + HD]
        wr[:, hb + HD // 2 : hb + HD] = w[:, hb : hb + HD // 2]
    return wr


def _to_bf16(a):
    return np.asarray(a, dtype=np.float32).astype(ml_dtypes.bfloat16)


def host_prep(inputs):
    tokens = np.clip(np.asarray(inputs["tokens"]), 0, 255).astype(np.int64)
    emb = np.asarray(inputs["embed_table"], np.float32)
    attn_w = np.asarray(inputs["attn_norm_w"], np.float32)
    ffn_w = np.asarray(inputs["ffn_norm_w"], np.float32)
    fin_w = np.asarray(inputs["final_norm_w"], np.float32)
    wq = np.asarray(inputs["wq"], np.float32) * attn_w[:, :, None]
    wk = np.asarray(inputs["wk"], np.float32) * attn_w[:, :, None]
    wv = np.asarray(inputs["wv"], np.float32) * attn_w[:, :, None]
    wo = np.asarray(inputs["wo"], np.float32)
    w1 = np.asarray(inputs["w1"], np.float32) * ffn_w[:, :, None]
    w2 = np.asarray(inputs["w2"], np.float32) * ffn_w[:, :, None]
    w3 = np.asarray(inputs["w3"], np.float32)

    wqr = np.stack([_rot_cols(wq[l]) for l in range(L)])
    wkr = np.stack([_rot_cols(wk[l]) for l in range(L)])

    shared = {
        "fw": fin_w,
        "ident": _to_bf16(np.eye(P, dtype=np.float32)),
        "emb": _to_bf16(emb),
        "wq": _to_bf16(wq),
        "wqr": _to_bf16(wqr),
        "wk": _to_bf16(wk),
        "wkr": _to_bf16(wkr),
        "wv": _to_bf16(wv),
        "wo": _to_bf16(wo),
        "w1": _to_bf16(w1),
        "w2": _to_bf16(w2),
        "w3": _to_bf16(w3),
    }

    # rope tables (rows: 2 head-blocks of 64; same pattern for every head pair)
    inv = 1.0 / (10000.0 ** (np.arange(0, HD, 2, dtype=np.float64) / HD))  # (32,)
    in_maps = []
    for c in range(N_CORES):
        b, half = c // 2, c % 2
        tok = tokens[b, half * TL : (half + 1) * TL]
        posf = np.arange(T, dtype=np.float64)
        ff = posf[None, :] * inv[:, None]  # (32, T)
        cos32, sin32 = np.cos(ff), np.sin(ff)
        cos64 = np.concatenate([cos32, cos32], 0)  # (64, T)
        sin64 = np.concatenate([sin32, sin32], 0)
        cosF = np.concatenate([cos64, cos64], 0)  # (128, T)
        sinF = np.concatenate([sin64, sin64], 0)
        cosT = cosF[:, half * TL : (half + 1) * TL]
        sinT = sinF[:, half * TL : (half + 1) * TL]

        oet = np.zeros((256, TL), np.float32)
        oet[tok, np.arange(TL)] = 1.0

        # segments
        is_sep = SEP_TABLE[tokens[b]]
        seg = np.cumsum(is_sep.astype(np.int64))  # inclusive, full row
        cnt = np.bincount(seg, minlength=seg[-1] + 1).astype(np.float64)
        ids = seg[half * TL : (half + 1) * TL]
        base = ids[0]
        loc = ids - base
        S_loc = int(loc[-1]) + 1
        assert S_loc <= SEG, f"too many segments {S_loc}"
        og = np.zeros((TL, SEG), np.float32)
        og[np.arange(TL), loc] = 1.0
        icnt = np.ones(SEG, np.float64)
        icnt[:S_loc] = 0.5 / np.maximum(cnt[base : base + S_loc], 1.0)
        esnd = np.zeros(SEG, np.float32)
        wfxm = np.zeros((P, SEG), np.float32)
        obind = np.zeros((P, TL), np.float32)
        if seg[TL - 1] == seg[TL]:  # a segment spans the half boundary
            sb_loc = int(seg[TL] - base) if half == 1 else int(seg[TL - 1] - base)
            esnd[sb_loc] = 1.0
            wfxm[1 - half, sb_loc] = 1.0
            s_b = sb_loc + base
            obind[1 - half, :] = (ids == s_b) * (0.5 / max(cnt[s_b], 1.0))

        in_maps.append(
            dict(
                shared,
                oet=_to_bf16(oet),
                og=_to_bf16(og),
                otg=_to_bf16(og.T.copy()),
                icnt=icnt.astype(np.float32),
                wfx=wfxm,
                obind=_to_bf16(obind),
                esnd=_to_bf16(np.repeat(esnd[:, None], P, 1)),
                cos=_to_bf16(cosT),
                sin=_to_bf16(sinT),
                cosf=_to_bf16(cosF),
                sinf=_to_bf16(sinF),
            )
        )
    return in_maps


class Runner:
    """Compile once; keep inputs device-resident; re-upload only changed data."""

    def __init__(self, nc):
        import jax
        import jax.numpy as jnp
        from jax.experimental.shard_map import shard_map
        from jax.sharding import Mesh, PartitionSpec
        import concourse.mybir as mybir_
        from concourse import bass2jax

        bass2jax.install_neuronx_cc_hook()
        self.jax = jax
        self.nc = nc
        in_names, out_names, out_avals, zero_outs = [], [], [], []
        for alloc in nc.m.functions[0].allocations:
            if not isinstance(mybir_.MemoryLocationSet, type) or not isinstance(
                alloc, mybir_.MemoryLocationSet
            ):
                continue
            name = alloc.memorylocations[0].name
            if alloc.kind == "ExternalInput":
                if nc.partition_id_tensor is None or name != nc.partition_id_tensor.name:
                    in_names.append(name)
            elif alloc.kind == "ExternalOutput":
                shape = tuple(alloc.tensor_shape)
                dtype = mybir_.dt.np(alloc.dtype)
                out_names.append(name)
                out_avals.append(jax.core.ShapedArray(shape, dtype))
                zero_outs.append(np.zeros(shape, dtype))
        self.n_params = len(in_names)
        self.in_names = list(in_names)
        self.out_names = out_names
        all_in_names = in_names + out_names
        partition_name = nc.partition_id_tensor.name if nc.partition_id_tensor else None
        if partition_name is not None:
            all_in_names = all_in_names + [partition_name]

        def _body(*args):
            operands = list(args)
            if partition_name is not None:
                operands.append(bass2jax.partition_id_tensor())
            outs = bass2jax._bass_exec_p.bind(
                *operands,
                out_avals=tuple(out_avals),
                in_names=tuple(all_in_names),
                out_names=tuple(out_names),
                lowering_input_output_aliases=(),
                sim_require_finite=True,
                sim_require_nnan=True,
                nc=nc,
            )
            return tuple(outs)

        devices = jax.devices()[:N_CORES]
        mesh = Mesh(np.asarray(devices), ("core",))
        n_in = self.n_params + len(out_names)
        self.sharded = jax.jit(
            shard_map(
                _body,
                mesh=mesh,
                in_specs=(PartitionSpec("core"),) * n_in,
                out_specs=(PartitionSpec("core"),) * len(out_names),
                check_rep=False,
            ),
            keep_unused=True,
        )
        self.mesh = mesh
        self.zero_outs = zero_outs
        self._dev_zero = None
        self._cache_np = {}
        self._cache_dev = {}

    def _put(self, name, arrs):
        """Concat per-core numpy arrays and put sharded on device (cached)."""
        import jax
        from jax.sharding import NamedSharding, PartitionSpec

        cached = self._cache_np.get(name)
        if cached is not None and all(
            a is b or (a.shape == b.shape and np.array_equal(a, b))
            for a, b in zip(cached, arrs)
        ):
            return self._cache_dev[name]
        glob = np.concatenate([np.asarray(a) for a in arrs], axis=0)
        dev = jax.device_put(glob, NamedSharding(self.mesh, PartitionSpec("core")))
        self._cache_np[name] = [np.asarray(a) for a in arrs]
        self._cache_dev[name] = dev
        return dev

    def __call__(self, in_maps):
        import jax
        from jax.sharding import NamedSharding, PartitionSpec

        args = [
            self._put(name, [m[name] for m in in_maps]) for name in self.in_names
        ]
        if self._dev_zero is None:
            self._dev_zero = [
                jax.device_put(
                    np.zeros((N_CORES * z.shape[0], *z.shape[1:]), z.dtype),
                    NamedSharding(self.mesh, PartitionSpec("core")),
                )
                for z in self.zero_outs
            ]
        outs = self.sharded(*args, *self._dev_zero)
        outs = [np.asarray(o) for o in outs]
        return {
            name: outs[i].reshape(N_CORES, *self.zero_outs[i].shape)
            for i, name in enumerate(self.out_names)
        }


_RUNNER = None
_RUNNER_FLAGS = None


def _get_runner(apply_fw=False):
    global _RUNNER, _RUNNER_FLAGS
    if _RUNNER is None or _RUNNER_FLAGS != (apply_fw,):
        nc = build_program(apply_fw=apply_fw)
        _RUNNER = Runner(nc)
        _RUNNER_FLAGS = (apply_fw,)
    return _RUNNER


def kernel(**inputs):
    apply_fw = not np.allclose(np.asarray(inputs["final_norm_w"]), 1.0)
    runner = _get_runner(apply_fw=apply_fw)
    in_maps = host_prep(inputs)
    res = runner(in_maps)
    y = res["y"]
    out = np.zeros((B, T, D), np.float32)
    for c in range(N_CORES):
        b, half = c // 2, c % 2
        out[b, half * TL : (half + 1) * TL, :] = y[c]
    return out



# revision 14
# speedup vs baseline: 1.1720x; 1.1720x over previous
"""ByteContextEncoder Trainium2 kernel (v2 — collective-hidden attention).

8-core SPMD sharding: core c handles batch row c//2, sequence half c%2
(TL = 1024 tokens). Attention needs full-row K/V. Per layer the post-norm
hidden h is pair-AllGathered; while the collective runs, the core computes
q/k/v for its LOCAL tokens and runs the LOCAL half of attention
(scores+softmax-exp+partial av against local keys, partial av spilled to
SBUF). After the collective lands, the partner half h is selected into a
fixed "remote" slot (SPMD-uniform via a per-core 0/1 mask input), k/v for
the remote keys are projected, and the REMOTE attention wave completes the
softmax average. The remote wave is emitted qb-major so attention output
tiles finish early and the FFN is pipelined behind it per 512-token half.

Segment mean pooling uses one-hot gather/scatter matmuls; the one segment
that can span the half boundary is fixed up on the HOST (each core outputs
its per-segment sums; the rank-1 cross-half correction is a tiny numpy
update), removing the tail collective entirely.

All float tensor compute runs on device in bf16 (f32 residual/psum).
Host only builds index structures (one-hot matrices, rope tables, counts)
derived from the integer `tokens` input, and casts weights to bf16.
"""

import math

import numpy as np
import ml_dtypes

import concourse.bass as bass
import concourse.mybir as mybir
import concourse.tile as tile

BF16 = mybir.dt.bfloat16
F32 = mybir.dt.float32
AX = mybir.AxisListType
ALU = mybir.AluOpType
ACT = mybir.ActivationFunctionType

# model dims (hardcoded per problem spec)
B, T, D, H, L = 4, 2048, 512, 8, 2
FF = 4 * D
HD = D // H
EPS = 1e-6
ALPHA = 0.5

N_CORES = 8

P = 128
TL = T // 2          # tokens per core
NT = TL // P         # 8 local token tiles
DC = D // P          # 4 D chunks
FFC = FF // P        # 16 FF chunks
SEG = 384            # padded segments per core
SC = SEG // P        # 3 segment chunks
QBW = 256            # q-block width for attention
NQB = TL // QBW      # 4 q blocks
NKT = TL // P        # 8 k tiles per wave (local / remote)

_SEP = b" \t\n\r.,;:!?()[]{}\"'" + b"+-*/=<>|&^~%@#$\\"
SEP_TABLE = np.zeros(256, dtype=bool)
SEP_TABLE[list(_SEP)] = True


def split_multiwait_drains(nc, max_waits=1):
    """This container's walrus can't encode >1 sync-wait on an instruction;
    hoist extra waits onto single-wait NoOps just before it (same engine, so
    sequencer order preserves the wait-before-execute semantics)."""
    n_patched = 0
    for f in nc.m.functions:
        for bb in f.blocks:
            new_list = []
            changed = False
            for ins in bb.instructions:
                si = ins.sync_info
                if si is not None and si.on_wait and len(si.on_wait) > max_waits:
                    for k, w in enumerate(si.on_wait):
                        nop = mybir.InstNoOp(name=f"{ins.name}-w{k}", ins=[], outs=[])
                        nop.engine = ins.engine
                        nop.sync_info = mybir.SyncInfo(on_wait=[w], on_update=[])
                        new_list.append(nop)
                    ins.sync_info = mybir.SyncInfo(
                        on_wait=[], on_update=list(si.on_update)
                    )
                    changed = True
                    n_patched += 1
                new_list.append(ins)
            if changed:
                bb.instructions = new_list
    return n_patched


def build_program(debug=(), patch=True, stage=5, for_sim=False, apply_fw=False):
    nc = bass.Bass(num_devices=N_CORES)

    # ---------------- DRAM inputs ----------------
    d_emb = nc.dram_tensor("emb", [256, D], BF16, kind="ExternalInput")
    d_ident = nc.dram_tensor("ident", [P, P], BF16, kind="ExternalInput")
    d_oet = nc.dram_tensor("oet", [256, TL], BF16, kind="ExternalInput")
    d_og = nc.dram_tensor("og", [TL, SEG], BF16, kind="ExternalInput")
    d_otg = nc.dram_tensor("otg", [SEG, TL], BF16, kind="ExternalInput")
    d_icnt = nc.dram_tensor("icnt", [SEG], F32, kind="ExternalInput")
    d_selm = nc.dram_tensor("selm", [P, 2], F32, kind="ExternalInput")
    d_fw = nc.dram_tensor("fw", [D], F32, kind="ExternalInput")
    d_cos2 = nc.dram_tensor("cos2", [P, 2, TL], BF16, kind="ExternalInput")
    d_sin2 = nc.dram_tensor("sin2", [P, 2, TL], BF16, kind="ExternalInput")
    d_wq = nc.dram_tensor("wq", [L, D, D], BF16, kind="ExternalInput")
    d_wqr = nc.dram_tensor("wqr", [L, D, D], BF16, kind="ExternalInput")
    d_wk = nc.dram_tensor("wk", [L, D, D], BF16, kind="ExternalInput")
    d_wkr = nc.dram_tensor("wkr", [L, D, D], BF16, kind="ExternalInput")
    d_wv = nc.dram_tensor("wv", [L, D, D], BF16, kind="ExternalInput")
    d_wo = nc.dram_tensor("wo", [L, D, D], BF16, kind="ExternalInput")
    d_w1 = nc.dram_tensor("w1", [L, D, FF], BF16, kind="ExternalInput")
    d_w2 = nc.dram_tensor("w2", [L, D, FF], BF16, kind="ExternalInput")
    d_w3 = nc.dram_tensor("w3", [L, FF, D], BF16, kind="ExternalInput")

    d_y = nc.dram_tensor("y", [TL, D], F32, kind="ExternalOutput")
    d_ss = nc.dram_tensor("ss", [SEG, D], BF16, kind="ExternalOutput")
    dbg_out = {}

    def dbg(name, shape, dtype=F32):
        if name in debug:
            dbg_out[name] = nc.dram_tensor(
                "dbg_" + name, shape, dtype, kind="ExternalOutput"
            )
            return dbg_out[name]
        return None

    with tile.TileContext(nc) as tc:
        with (
            tc.tile_pool(name="state", bufs=1) as state,
            tc.tile_pool(name="aux", bufs=1) as aux,
            tc.tile_pool(name="wsm", bufs=1) as wsm,
            tc.tile_pool(name="wff", bufs=4) as wff,
            tc.tile_pool(name="stp", bufs=2) as stp,
            tc.tile_pool(name="tmp", bufs=3) as tmp,
            tc.tile_pool(name="psum", bufs=4, space="PSUM") as psum,
            tc.tile_pool(name="psum_st", bufs=2, space="PSUM") as psum_st,
            tc.tile_pool(name="dram", bufs=1, space="DRAM") as dram,
        ):
            # ---- persistent state ----
            x_sb = state.tile([P, NT, D], F32, tag="x")          # residual
            cos2 = state.tile([P, 2, TL], BF16, tag="cos2")
            sin2 = state.tile([P, 2, TL], BF16, tag="sin2")
            ident = state.tile([P, P], BF16, tag="ident")
            selm = state.tile([P, 2], F32, tag="selm")
            eps_sb = state.tile([P, 1], F32, tag="eps")
            nc.vector.memset(eps_sb[:], EPS)

            # h (both halves, transposed): [mc, {0:local,1:remote}, tok]
            hTf = state.tile([P, DC, 2, TL], BF16, tag="hTf")
            # k (rope'd): A = heads 0-3 (mc pairs 0,1), B = heads 4-7
            kTfA = state.tile([P, 2, 2, TL], BF16, tag="kTfA")
            kTfB = state.tile([P, 2, 2, TL], BF16, tag="kTfB")
            # v (+ ones col): [w, kt, h, hd+1]
            vf = state.tile([P, 2, NKT, H, HD + 1], BF16, tag="vf")
            qT = state.tile([P, DC, TL], BF16, tag="qT")
            # local-wave av partial spill (bf16: [h, q-tile, hd+1])
            avp = state.tile([P, H, NT, HD + 1], BF16, tag="avp")

            # ---- embedding: x = onehot @ table (inputs loaded first) ----
            embt = aux.tile([P, 2, D], BF16, tag="embt")
            oet = stp.tile([P, 2, TL], BF16, tag="st")
            nc.sync.dma_start(oet[:], d_oet.rearrange("(c p) t -> p c t", p=P))
            nc.sync.dma_start(embt[:], d_emb.rearrange("(c p) d -> p c d", p=P))
            nc.sync.dma_start(ident[:], d_ident[:])
            nc.sync.dma_start(cos2[:], d_cos2[:])
            nc.sync.dma_start(sin2[:], d_sin2[:])
            nc.sync.dma_start(selm[:], d_selm[:])
            for t in range(NT):
                ps = psum.tile([P, 512], F32, tag="mm")
                for kc in range(2):
                    nc.tensor.matmul(
                        ps[:],
                        oet[:, kc, t * P : (t + 1) * P],
                        embt[:, kc, :],
                        start=(kc == 0),
                        stop=(kc == 1),
                    )
                nc.vector.tensor_copy(x_sb[:, t, :], ps[:])

            def rmsnorm_tile(t, out_tile, out_slice):
                """out = x_sb[:,t,:] * rsqrt(mean(x^2)+eps), bf16."""
                xsq = tmp.tile([P, D], BF16, tag="h")
                ssq = tmp.tile([P, 1], F32, tag="ssq")
                nc.scalar.activation(
                    xsq[:], x_sb[:, t, :], ACT.Square, accum_out=ssq[:]
                )
                nc.scalar.activation(
                    ssq[:], ssq[:], ACT.Sqrt, bias=eps_sb[:], scale=1.0 / D
                )
                nc.vector.reciprocal(ssq[:], ssq[:])
                nc.vector.tensor_scalar_mul(out_tile[out_slice], x_sb[:, t, :], ssq[:])

            def transpose_block(dst_ap, src_ap, use_act=False):
                """[128,128] bf16 transpose via PE; psum->sbuf copy."""
                pt = psum.tile([P, P], BF16, tag="mm", name="pt")
                nc.tensor.transpose(pt[:], src_ap, ident[:])
                if use_act:
                    nc.scalar.copy(dst_ap, pt[:])
                else:
                    nc.vector.tensor_copy(dst_ap, pt[:])

            # pooling index matrices: issue on the idle Act DMA queue so they
            # don't delay weight loads on the SP queue
            og_e = aux.tile([P, NT, SEG], BF16, tag="og")
            otg_e = aux.tile([P, SC, TL], BF16, tag="otg")
            icnt_e = aux.tile([P, SC, 1], F32, tag="icnt")
            nc.scalar.dma_start(og_e[:], d_og.rearrange("(c p) s -> p c s", p=P))
            nc.scalar.dma_start(otg_e[:], d_otg.rearrange("(c p) t -> p c t", p=P))
            nc.scalar.dma_start(
                icnt_e[:], d_icnt.rearrange("(c p) -> p c", p=P)[:, :, None]
            )

            def prologue(l, h_all):
                """rmsnorm + transpose local h into hTf, DMA to bh_in, start
                the pair-AllGather. Returns (bh_out,)."""
                for t in range(NT):
                    rmsnorm_tile(t, h_all, np.s_[:, t, :])
                bh_in = dram.tile([D * TL], BF16, tag="bkv_in")
                bh_out = dram.tile([2 * D * TL], BF16, tag="bkv_out")
                for mc in range(DC):
                    for t in range(NT):
                        transpose_block(
                            hTf[:, mc, 0, t * P : (t + 1) * P],
                            h_all[:, t, mc * P : (mc + 1) * P],
                            use_act=True,
                        )
                    nc.sync.dma_start(
                        bh_in[mc * P * TL : (mc + 1) * P * TL].rearrange(
                            "(p x) -> p x", p=P
                        ),
                        hTf[:, mc, 0, :],
                    )
                nc.gpsimd.collective_compute(
                    "AllGather",
                    ALU.bypass,
                    replica_groups=[[0, 1], [2, 3], [4, 5], [6, 7]],
                    ins=[bh_in[:].opt()],
                    outs=[bh_out[:].opt()],
                )
                return bh_out

            def proj_rope(w_sb, wr_sb, out_tile, out_pre, w, cos_t, sin_t):
                """rope'd projection of hTf[:, :, w, :] -> out_tile.

                out_pre(mc, nh) -> (ap_a, ap_b, out_ap) selects psum/psum
                shapes and destination slices; nh iterates 512-wide chunks of
                TL."""
                for mc in range(DC):
                    for nh in range(TL // 512):
                        nsl = np.s_[nh * 512 : (nh + 1) * 512]
                        pa = psum.tile([P, 512], F32, tag="mm", name="pa")
                        pb = psum.tile([P, 512], F32, tag="mm", name="pb")
                        for kc in range(DC):
                            nc.tensor.matmul(
                                pa[:],
                                w_sb[:, kc, mc * P : (mc + 1) * P],
                                hTf[:, kc, w, nsl],
                                start=(kc == 0),
                                stop=(kc == DC - 1),
                            )
                        for kc in range(DC):
                            nc.tensor.matmul(
                                pb[:],
                                wr_sb[:, kc, mc * P : (mc + 1) * P],
                                hTf[:, kc, w, nsl],
                                start=(kc == 0),
                                stop=(kc == DC - 1),
                            )
                        t1 = tmp.tile([P, 512], BF16, tag="rope")
                        t2 = tmp.tile([P, 512], BF16, tag="rope")
                        nc.vector.tensor_mul(t1[:], pa[:], cos_t[:, nsl])
                        nc.vector.tensor_mul(t2[:], pb[:], sin_t[:, nsl])
                        nc.vector.tensor_add(out_pre(mc, nh), t1[:], t2[:])

            def v_proj(wv_sb, w):
                for t in range(NKT):
                    ps = psum.tile([P, 512], F32, tag="mm")
                    for kc in range(DC):
                        nc.tensor.matmul(
                            ps[:],
                            hTf[:, kc, w, t * P : (t + 1) * P],
                            wv_sb[:, kc, :],
                            start=(kc == 0),
                            stop=(kc == DC - 1),
                        )
                    nc.vector.tensor_copy(
                        vf[:, w, t, :, :HD], ps[:].rearrange("p (h d) -> p h d", h=H)
                    )
                    nc.vector.memset(vf[:, w, t, :, HD], 1.0)

            def emit_scores(h, qb, w):
                """scores + exp for block (head h, q-block qb) vs wave-w keys.
                Returns st tile [P, NKT, QBW]."""
                po = (h % 2) * HD
                hc = h // 2
                ksrc = kTfA if hc < 2 else kTfB
                qsl = np.s_[qb * QBW : (qb + 1) * QBW]
                st_sb = stp.tile([P, 2, NKT // 2, QBW], BF16, tag="st", name="st_sb")
                for k4 in range(NKT // 4):
                    pst = psum_st.tile([P, 4, QBW], F32, tag="st")
                    for j in range(4):
                        kt = k4 * 4 + j
                        nc.tensor.matmul(
                            pst[:, j, :],
                            ksrc[po : po + HD, hc % 2, w, kt * P : (kt + 1) * P],
                            qT[po : po + HD, hc, qsl],
                            start=True,
                            stop=True,
                        )
                    nc.scalar.activation(
                        st_sb.rearrange("p a b q -> p (a b) q")[
                            :, k4 * 4 : (k4 + 1) * 4, :
                        ],
                        pst[:],
                        ACT.Exp,
                        scale=1.0 / math.sqrt(HD),
                    )
                return st_sb.rearrange("p a b q -> p (a b) q")

            def emit_av_local(h, qb, st):
                for qt in range(QBW // P):
                    tix = qb * (QBW // P) + qt
                    pav = psum.tile([P, 512], F32, tag="mm", name="pav")
                    for kt in range(NKT):
                        nc.tensor.matmul(
                            pav[:, : HD + 1],
                            st[:, kt, qt * P : (qt + 1) * P],
                            vf[:, 0, kt, h, :],
                            start=(kt == 0),
                            stop=(kt == NKT - 1),
                        )
                    nc.vector.tensor_copy(avp[:, h, tix, :], pav[:, : HD + 1])

            def emit_av_remote(h, qb, st, o_sb):
                for qt in range(QBW // P):
                    tix = qb * (QBW // P) + qt
                    pav = psum.tile([P, 512], F32, tag="mm", name="pav")
                    for kt in range(NKT):
                        nc.tensor.matmul(
                            pav[:, : HD + 1],
                            st[:, kt, qt * P : (qt + 1) * P],
                            vf[:, 1, kt, h, :],
                            start=(kt == 0),
                            stop=(kt == NKT - 1),
                        )
                    avf = tmp.tile([P, HD + 1], F32, tag="avf")
                    nc.vector.tensor_add(avf[:], pav[:, : HD + 1], avp[:, h, tix, :])
                    rcp = tmp.tile([P, 1], F32, tag="rcp")
                    nc.vector.reciprocal(rcp[:], avf[:, HD : HD + 1])
                    nc.vector.tensor_scalar_mul(
                        o_sb[:, tix, h * HD : (h + 1) * HD], avf[:, :HD], rcp[:]
                    )

            # ================= layers =================
            h_all = stp.tile([P, NT, D], BF16, tag="o_or_h", bufs=1)
            o_sb = h_all  # same storage: h_all consumed before o is written
            for l in range(L if stage >= 2 else 1):
                # weight DMAs issued before the prologue's bh_in DMAs so the
                # in-order SP queue doesn't delay them behind the transposes
                w_sb = wsm.tile([P, DC, D], BF16, tag="wq")
                wr_sb = wsm.tile([P, DC, D], BF16, tag="wqr")
                wk_sb = wsm.tile([P, DC, D], BF16, tag="wk")
                wkr_sb = wsm.tile([P, DC, D], BF16, tag="wkr")
                wv_sb = wsm.tile([P, DC, D], BF16, tag="wv")
                nc.sync.dma_start(w_sb[:], d_wq[l].rearrange("(c p) n -> p c n", p=P))
                nc.sync.dma_start(wr_sb[:], d_wqr[l].rearrange("(c p) n -> p c n", p=P))
                nc.sync.dma_start(wk_sb[:], d_wk[l].rearrange("(c p) n -> p c n", p=P))
                nc.sync.dma_start(wkr_sb[:], d_wkr[l].rearrange("(c p) n -> p c n", p=P))
                nc.sync.dma_start(wv_sb[:], d_wv[l].rearrange("(c p) n -> p c n", p=P))

                bh_out = prologue(l, h_all)

                # ---- q projection + rope from local h (overlaps collective)
                proj_rope(
                    w_sb, wr_sb, qT, lambda mc, nh: qT[:, mc, nh * 512 : (nh + 1) * 512],
                    0, cos2[:, 0, :], sin2[:, 0, :],
                )

                def k_out(w):
                    def _f(mc, nh):
                        ktile = kTfA if mc < 2 else kTfB
                        return ktile[:, mc % 2, w, nh * 512 : (nh + 1) * 512]
                    return _f

                proj_rope(wk_sb, wkr_sb, None, k_out(0), 0,
                          cos2[:, 0, :], sin2[:, 0, :])

                # ---- v local ----
                v_proj(wv_sb, 0)

                if stage < 3:
                    break

                def emit_select():
                    # select partner half h into hTf[:, :, 1, :]; emitted
                    # late in the local wave so the collective has landed and
                    # the DVE ops run in the wave's shadow
                    for mc in range(DC):
                        hst = stp.tile([P, 2, TL], BF16, tag="st", name="hst")
                        for r in range(2):
                            nc.sync.dma_start(
                                hst[:, r, :],
                                bh_out[r * D * TL + mc * P * TL :][
                                    : P * TL
                                ].rearrange("(p x) -> p x", p=P),
                            )
                        hsel = tmp.tile([P, TL], BF16, tag="hsel", bufs=1)
                        nc.vector.tensor_scalar_mul(
                            hsel[:], hst[:, 0, :], selm[:, 0:1]
                        )
                        nc.vector.scalar_tensor_tensor(
                            hTf[:, mc, 1, :], hst[:, 1, :], selm[:, 1:2], hsel[:],
                            op0=ALU.mult, op1=ALU.add,
                        )

                # ---- local attention wave (overlaps collective) ----
                prev = None
                for h in range(H):
                    for qb in range(NQB):
                        st = emit_scores(h, qb, 0)
                        if prev is not None:
                            emit_av_local(*prev)
                        prev = (h, qb, st)
                    if h == H - 2:
                        emit_select()
                emit_av_local(*prev)

                # ---- k/v remote ----
                proj_rope(wk_sb, wkr_sb, None, k_out(1), 1,
                          cos2[:, 1, :], sin2[:, 1, :])
                v_proj(wv_sb, 1)

                # ---- remote attention wave, qb-major; wo + FFN pipelined ----
                wo_sb = wsm.tile([P, DC, D], BF16, tag="wo")
                nc.sync.dma_start(wo_sb[:], d_wo[l].rearrange("(c p) n -> p c n", p=P))
                w3_sb = wsm.tile([P, FFC, D], BF16, tag="w3")
                nc.sync.dma_start(w3_sb[:], d_w3[l].rearrange("(c p) n -> p c n", p=P))

                oT = state.tile([P, DC, 2, P], BF16, tag="oT")  # per-qb transposed o
                hT2 = state.tile([P, DC, 512], BF16, tag="hT2")  # per-th ffn input

                def ffn_half(th):
                    # rmsnorm + transpose the 4 fresh x tiles
                    for t in range(th * 4, th * 4 + 4):
                        h2 = tmp.tile([P, D], BF16, tag="h")
                        rmsnorm_tile(t, h2, np.s_[:])
                        for mc in range(DC):
                            transpose_block(
                                hT2[:, mc, (t - th * 4) * P : (t - th * 4 + 1) * P],
                                h2[:, mc * P : (mc + 1) * P],
                            )
                    h12 = state.tile([P, FFC, 512], BF16, tag="h12")
                    for mc2 in range(FFC // 2):
                        msl = np.s_[:, mc2 * 2 * P : (mc2 + 1) * 2 * P]
                        w1c = wff.tile([P, DC, 2 * P], BF16, tag="wffc")
                        w2c = wff.tile([P, DC, 2 * P], BF16, tag="wffc")
                        nc.sync.dma_start(
                            w1c[:], d_w1[l][msl].rearrange("(c p) n -> p c n", p=P)
                        )
                        nc.sync.dma_start(
                            w2c[:], d_w2[l][msl].rearrange("(c p) n -> p c n", p=P)
                        )
                        for mi in range(2):
                            mc = mc2 * 2 + mi
                            p1 = psum.tile([P, 512], F32, tag="mm", name="p1")
                            p2 = psum.tile([P, 512], F32, tag="mm", name="p2")
                            for kc in range(DC):
                                nc.tensor.matmul(
                                    p1[:],
                                    w1c[:, kc, mi * P : (mi + 1) * P],
                                    hT2[:, kc, :],
                                    start=(kc == 0),
                                    stop=(kc == DC - 1),
                                )
                            for kc in range(DC):
                                nc.tensor.matmul(
                                    p2[:],
                                    w2c[:, kc, mi * P : (mi + 1) * P],
                                    hT2[:, kc, :],
                                    start=(kc == 0),
                                    stop=(kc == DC - 1),
                                )
                            sl = tmp.tile([P, 512], BF16, tag="h")
                            if for_sim:
                                nc.scalar.activation(sl[:], p1[:], ACT.Sigmoid)
                                u = tmp.tile([P, 512], BF16, tag="h")
                                nc.vector.tensor_mul(u[:], p1[:], sl[:])
                                nc.vector.tensor_mul(h12[:, mc, :], p2[:], u[:])
                            else:
                                nc.scalar.activation(sl[:], p1[:], ACT.Silu)
                                nc.vector.tensor_mul(h12[:, mc, :], p2[:], sl[:])
                    for t in range(th * 4, th * 4 + 4):
                        ps = psum.tile([P, 512], F32, tag="mm")
                        for kc in range(FFC):
                            nc.tensor.matmul(
                                ps[:],
                                h12[:, kc, (t - th * 4) * P : (t - th * 4 + 1) * P],
                                w3_sb[:, kc, :],
                                start=(kc == 0),
                                stop=(kc == FFC - 1),
                            )
                        nc.vector.tensor_add(x_sb[:, t, :], x_sb[:, t, :], ps[:])

                prev = None
                for qb in range(NQB):
                    for h in range(H):
                        st = emit_scores(h, qb, 1)
                        if prev is not None:
                            emit_av_remote(*prev, o_sb)
                        prev = (h, qb, st)
                    emit_av_remote(*prev, o_sb)
                    prev = None
                    # o[qb] complete -> transpose + wo + residual
                    for qt in range(QBW // P):
                        tix = qb * (QBW // P) + qt
                        for mc in range(DC):
                            transpose_block(
                                oT[:, mc, qt, :],
                                o_sb[:, tix, mc * P : (mc + 1) * P],
                            )
                        ps = psum.tile([P, 512], F32, tag="mm")
                        for mc in range(DC):
                            nc.tensor.matmul(
                                ps[:],
                                oT[:, mc, qt, :],
                                wo_sb[:, mc, :],
                                start=(mc == 0),
                                stop=(mc == DC - 1),
                            )
                        nc.vector.tensor_add(x_sb[:, tix, :], x_sb[:, tix, :], ps[:])
                    if stage >= 4 and qb % 2 == 1:
                        ffn_half(qb // 2)

            # ================= final norm + segment pooling =================
            xn = o_sb  # reuse storage
            for t in range(NT):
                rmsnorm_tile(t, xn, np.s_[:, t, :])

            if stage < 5:
                for t in range(NT):
                    xf = tmp.tile([P, D], F32, tag="f32t", bufs=2)
                    nc.vector.tensor_copy(xf[:], xn[:, t, :])
                    nc.sync.dma_start(
                        d_y.rearrange("(t p) d -> t p d", p=P)[t], xf[:]
                    )
                zss = tmp.tile([P, SC, D], BF16, tag="segsum", bufs=1)
                nc.vector.memset(zss[:], 0.0)
                nc.sync.dma_start(d_ss.rearrange("(c p) d -> p c d", p=P), zss[:])
            if stage >= 5:
                if apply_fw:
                    fw_bc = aux.tile([P, D], F32, tag="fw_bc")
                    nc.sync.dma_start(
                        fw_bc[:],
                        bass.AP(tensor=d_fw, offset=0, ap=[[0, P], [1, D]]),
                    )

                segsum_bf = aux.tile([P, SC, D], BF16, tag="segsum_bf")
                for mc in range(SC):
                    ps = psum.tile([P, 512], F32, tag="mm")
                    for kt in range(NT):
                        nc.tensor.matmul(
                            ps[:],
                            og_e[:, kt, mc * P : (mc + 1) * P],
                            xn[:, kt, :],
                            start=(kt == 0),
                            stop=(kt == NT - 1),
                        )
                    nc.scalar.copy(segsum_bf[:, mc, :], ps[:])
                # per-segment sums out for the host-side boundary fix
                nc.sync.dma_start(
                    d_ss.rearrange("(c p) d -> p c d", p=P), segsum_bf[:]
                )

                # seg_mean = partial * icnt (icnt = 0.5/cnt_total); the
                # cross-half contribution for the one boundary segment is
                # added on the host from the ss outputs.
                segmean = aux.tile([P, SC, D], BF16, tag="segmean")
                for mc in range(SC):
                    nc.vector.tensor_scalar_mul(
                        segmean[:, mc, :], segsum_bf[:, mc, :], icnt_e[:, mc, :]
                    )

                # out = 0.5*xn + scatter(segmean)
                for t in range(NT):
                    ps = psum.tile([P, 512], F32, tag="mm")
                    for kc in range(SC):
                        nc.tensor.matmul(
                            ps[:],
                            otg_e[:, kc, t * P : (t + 1) * P],
                            segmean[:, kc, :],
                            start=(kc == 0),
                            stop=(kc == SC - 1),
                        )
                    out_t = tmp.tile([P, D], F32, tag="f32t", bufs=2)
                    nc.vector.scalar_tensor_tensor(
                        out_t[:], xn[:, t, :], ALPHA, ps[:],
                        op0=ALU.mult, op1=ALU.add,
                    )
                    if apply_fw:
                        nc.vector.tensor_mul(out_t[:], out_t[:], fw_bc[:])
                    nc.sync.dma_start(
                        d_y.rearrange("(t p) d -> t p d", p=P)[t], out_t[:]
                    )

            # ---- debug taps ----
            if "x0" in debug:
                nc.sync.dma_start(
                    dbg_out["x0"].rearrange("(t p) d -> t p d", p=P)[:], x_sb[:]
                )

    if patch:
        split_multiwait_drains(nc)
    return nc


# ----------------------------------------------------------------------------
# host side
# ----------------------------------------------------------------------------


def _rot_cols(w):
    """Columns permuted/negated so (h @ w_rot) == rotate_half(h @ w)."""
    wr = np.empty_like(w)
    for hb in range(0, D, HD):
        wr[:, hb : hb + HD // 2] = -w[:, hb + HD // 2 : hb + HD]
        wr[:, hb + HD // 2 : hb + HD] = w[:, hb : hb + HD // 2]
    return wr


def _to_bf16(a):
    return np.asarray(a, dtype=np.float32).astype(ml_dtypes.bfloat16)


def host_prep(inputs):
    tokens = np.clip(np.asarray(inputs["tokens"]), 0, 255).astype(np.int64)
    emb = np.asarray(inputs["embed_table"], np.float32)
    attn_w = np.asarray(inputs["attn_norm_w"], np.float32)
    ffn_w = np.asarray(inputs["ffn_norm_w"], np.float32)
    fin_w = np.asarray(inputs["final_norm_w"], np.float32)
    wq = np.asarray(inputs["wq"], np.float32) * attn_w[:, :, None]
    wk = np.asarray(inputs["wk"], np.float32) * attn_w[:, :, None]
    wv = np.asarray(inputs["wv"], np.float32) * attn_w[:, :, None]
    wo = np.asarray(inputs["wo"], np.float32)
    w1 = np.asarray(inputs["w1"], np.float32) * ffn_w[:, :, None]
    w2 = np.asarray(inputs["w2"], np.float32) * ffn_w[:, :, None]
    w3 = np.asarray(inputs["w3"], np.float32)

    wqr = np.stack([_rot_cols(wq[l]) for l in range(L)])
    wkr = np.stack([_rot_cols(wk[l]) for l in range(L)])

    shared = {
        "fw": fin_w,
        "ident": _to_bf16(np.eye(P, dtype=np.float32)),
        "emb": _to_bf16(emb),
        "wq": _to_bf16(wq),
        "wqr": _to_bf16(wqr),
        "wk": _to_bf16(wk),
        "wkr": _to_bf16(wkr),
        "wv": _to_bf16(wv),
        "wo": _to_bf16(wo),
        "w1": _to_bf16(w1),
        "w2": _to_bf16(w2),
        "w3": _to_bf16(w3),
    }

    # rope tables (rows: 2 head-blocks of 64; same pattern for every head pair)
    inv = 1.0 / (10000.0 ** (np.arange(0, HD, 2, dtype=np.float64) / HD))  # (32,)
    in_maps = []
    for c in range(N_CORES):
        b, half = c // 2, c % 2
        tok = tokens[b, half * TL : (half + 1) * TL]
        posf = np.arange(T, dtype=np.float64)
        ff = posf[None, :] * inv[:, None]  # (32, T)
        cos32, sin32 = np.cos(ff), np.sin(ff)
        cos64 = np.concatenate([cos32, cos32], 0)  # (64, T)
        sin64 = np.concatenate([sin32, sin32], 0)
        cosF = np.concatenate([cos64, cos64], 0)  # (128, T)
        sinF = np.concatenate([sin64, sin64], 0)
        loc_sl = np.s_[:, half * TL : (half + 1) * TL]
        rem_sl = np.s_[:, (1 - half) * TL : (2 - half) * TL]
        cos2 = np.stack([cosF[loc_sl], cosF[rem_sl]], axis=1)  # (128, 2, TL)
        sin2 = np.stack([sinF[loc_sl], sinF[rem_sl]], axis=1)

        selm = np.zeros((P, 2), np.float32)
        selm[:, 1 - half] = 1.0  # partner occupies row (1-half) of bh_out

        oet = np.zeros((256, TL), np.float32)
        oet[tok, np.arange(TL)] = 1.0

        # segments
        is_sep = SEP_TABLE[tokens[b]]
        seg = np.cumsum(is_sep.astype(np.int64))  # inclusive, full row
        cnt = np.bincount(seg, minlength=seg[-1] + 1).astype(np.float64)
        ids = seg[half * TL : (half + 1) * TL]
        base = ids[0]
        loc = ids - base
        S_loc = int(loc[-1]) + 1
        assert S_loc <= SEG, f"too many segments {S_loc}"
        og = np.zeros((TL, SEG), np.float32)
        og[np.arange(TL), loc] = 1.0
        icnt = np.ones(SEG, np.float64)
        icnt[:S_loc] = 0.5 / np.maximum(cnt[base : base + S_loc], 1.0)

        in_maps.append(
            dict(
                shared,
                oet=_to_bf16(oet),
                og=_to_bf16(og),
                otg=_to_bf16(og.T.copy()),
                icnt=icnt.astype(np.float32),
                selm=selm,
                cos2=_to_bf16(cos2),
                sin2=_to_bf16(sin2),
            )
        )
    return in_maps


class Runner:
    """Compile once; keep inputs device-resident; re-upload only changed data."""

    def __init__(self, nc):
        import jax
        import jax.numpy as jnp
        from jax.experimental.shard_map import shard_map
        from jax.sharding import Mesh, PartitionSpec
        import concourse.mybir as mybir_
        from concourse import bass2jax

        bass2jax.install_neuronx_cc_hook()
        self.jax = jax
        self.nc = nc
        in_names, out_names, out_avals, zero_outs = [], [], [], []
        for alloc in nc.m.functions[0].allocations:
            if not isinstance(mybir_.MemoryLocationSet, type) or not isinstance(
                alloc, mybir_.MemoryLocationSet
            ):
                continue
            name = alloc.memorylocations[0].name
            if alloc.kind == "ExternalInput":
                if nc.partition_id_tensor is None or name != nc.partition_id_tensor.name:
                    in_names.append(name)
            elif alloc.kind == "ExternalOutput":
                shape = tuple(alloc.tensor_shape)
                dtype = mybir_.dt.np(alloc.dtype)
                out_names.append(name)
                out_avals.append(jax.core.ShapedArray(shape, dtype))
                zero_outs.append(np.zeros(shape, dtype))
        self.n_params = len(in_names)
        self.in_names = list(in_names)
        self.out_names = out_names
        all_in_names = in_names + out_names
        partition_name = nc.partition_id_tensor.name if nc.partition_id_tensor else None
        if partition_name is not None:
            all_in_names = all_in_names + [partition_name]

        def _body(*args):
            operands = list(args)
            if partition_name is not None:
                operands.append(bass2jax.partition_id_tensor())
            outs = bass2jax._bass_exec_p.bind(
                *operands,
                out_avals=tuple(out_avals),
                in_names=tuple(all_in_names),
                out_names=tuple(out_names),
                lowering_input_output_aliases=(),
                sim_require_finite=True,
                sim_require_nnan=True,
                nc=nc,
            )
            return tuple(outs)

        devices = jax.devices()[:N_CORES]
        mesh = Mesh(np.asarray(devices), ("core",))
        n_in = self.n_params + len(out_names)
        self.sharded = jax.jit(
            shard_map(
                _body,
                mesh=mesh,
                in_specs=(PartitionSpec("core"),) * n_in,
                out_specs=(PartitionSpec("core"),) * len(out_names),
                check_rep=False,
            ),
            keep_unused=True,
        )
        self.mesh = mesh
        self.zero_outs = zero_outs
        self._dev_zero = None
        self._cache_np = {}
        self._cache_dev = {}

    def _put(self, name, arrs):
        """Concat per-core numpy arrays and put sharded on device (cached)."""
        import jax
        from jax.sharding import NamedSharding, PartitionSpec

        cached = self._cache_np.get(name)
        if cached is not None and all(
            a is b or (a.shape == b.shape and np.array_equal(a, b))
            for a, b in zip(cached, arrs)
        ):
            return self._cache_dev[name]
        glob = np.concatenate([np.asarray(a) for a in arrs], axis=0)
        dev = jax.device_put(glob, NamedSharding(self.mesh, PartitionSpec("core")))
        self._cache_np[name] = [np.asarray(a) for a in arrs]
        self._cache_dev[name] = dev
        return dev

    def __call__(self, in_maps):
        import jax
        from jax.sharding import NamedSharding, PartitionSpec

        args = [
            self._put(name, [m[name] for m in in_maps]) for name in self.in_names
        ]
        if self._dev_zero is None:
            self._dev_zero = [
                jax.device_put(
                    np.zeros((N_CORES * z.shape[0], *z.shape[1:]), z.dtype),
                    NamedSharding(self.mesh, PartitionSpec("core")),
                )
                for z in self.zero_outs
            ]
        outs = self.sharded(*args, *self._dev_zero)
        outs = [np.asarray(o) for o in outs]
        return {
            name: outs[i].reshape(N_CORES, *self.zero_outs[i].shape)
            for i, name in enumerate(self.out_names)
        }


_RUNNER = None
_RUNNER_FLAGS = None


def _get_runner(apply_fw=False):
    global _RUNNER, _RUNNER_FLAGS
    if _RUNNER is None or _RUNNER_FLAGS != (apply_fw,):
        nc = build_program(apply_fw=apply_fw)
        _RUNNER = Runner(nc)
        _RUNNER_FLAGS = (apply_fw,)
    return _RUNNER


def kernel(**inputs):
    apply_fw = not np.allclose(np.asarray(inputs["final_norm_w"]), 1.0)
    runner = _get_runner(apply_fw=apply_fw)
    in_maps = host_prep(inputs)
    res = runner(in_maps)
    y = res["y"]
    ss = res["ss"]
    out = np.zeros((B, T, D), np.float32)
    for c in range(N_CORES):
        b, half = c // 2, c % 2
        out[b, half * TL : (half + 1) * TL, :] = y[c]

    # host-side cross-half boundary-segment correction
    tokens = np.clip(np.asarray(inputs["tokens"]), 0, 255).astype(np.int64)
    fin_w = np.asarray(inputs["final_norm_w"], np.float32)
    for b in range(B):
        is_sep = SEP_TABLE[tokens[b]]
        seg = np.cumsum(is_sep.astype(np.int64))
        if seg[TL - 1] != seg[TL]:
            continue  # separator right at the half boundary: no spanning seg
        s_b = int(seg[TL])
        cnt = np.bincount(seg, minlength=s_b + 1).astype(np.float64)
        corr = np.float32(0.5 / max(cnt[s_b], 1.0))
        cA, cB = 2 * b, 2 * b + 1
        baseA = int(seg[0])
        bpA = ss[cA][s_b - baseA].astype(np.float32)
        bpB = ss[cB][0].astype(np.float32)  # boundary seg is B's first segment
        if apply_fw:
            bpA = bpA * fin_w
            bpB = bpB * fin_w
        maskA = seg[:TL] == s_b
        maskB = seg[TL:] == s_b
        out[b, :TL][maskA] += corr * bpB[None, :]
        out[b, TL:][maskB] += corr * bpA[None, :]
    return out


# revision 42
# speedup vs baseline: 1.2347x; 1.0535x over previous
"""ByteContextEncoder Trainium2 kernel (v2 — collective-hidden attention).

8-core SPMD sharding: core c handles batch row c//2, sequence half c%2
(TL = 1024 tokens). Attention needs full-row K/V. Per layer the post-norm
hidden h is pair-AllGathered; while the collective runs, the core computes
q/k/v for its LOCAL tokens and runs the LOCAL half of attention
(scores+softmax-exp+partial av against local keys, partial av spilled to
SBUF). After the collective lands, the partner half h is selected into a
fixed "remote" slot (SPMD-uniform via a per-core 0/1 mask input), k/v for
the remote keys are projected, and the REMOTE attention wave completes the
softmax average. The remote wave is emitted qb-major so attention output
tiles finish early and the FFN is pipelined behind it per 512-token half.

Segment mean pooling uses one-hot gather/scatter matmuls; the one segment
that can span the half boundary is fixed up on the HOST (each core outputs
its per-segment sums; the rank-1 cross-half correction is a tiny numpy
update), removing the tail collective entirely.

All float tensor compute runs on device in bf16 (f32 residual/psum).
Host only builds index structures (one-hot matrices, rope tables, counts)
derived from the integer `tokens` input, and casts weights to bf16.
"""

import math

import numpy as np
import ml_dtypes

import concourse.bass as bass
import concourse.mybir as mybir
import concourse.tile as tile

BF16 = mybir.dt.bfloat16
F32 = mybir.dt.float32
AX = mybir.AxisListType
ALU = mybir.AluOpType
ACT = mybir.ActivationFunctionType

# model dims (hardcoded per problem spec)
B, T, D, H, L = 4, 2048, 512, 8, 2
FF = 4 * D
HD = D // H
EPS = 1e-6
ALPHA = 0.5

N_CORES = 8

P = 128
TL = T // 2          # tokens per core
NT = TL // P         # 8 local token tiles
DC = D // P          # 4 D chunks
FFC = FF // P        # 16 FF chunks
SEG = 384            # padded segments per core
SC = SEG // P        # 3 segment chunks
QBW = 256            # q-block width for attention
NQB = TL // QBW      # 4 q blocks
NKT = TL // P        # 8 k tiles per wave (local / remote)

_SEP = b" \t\n\r.,;:!?()[]{}\"'" + b"+-*/=<>|&^~%@#$\\"
SEP_TABLE = np.zeros(256, dtype=bool)
SEP_TABLE[list(_SEP)] = True


def split_multiwait_drains(nc, max_waits=1):
    """This container's walrus can't encode >1 sync-wait on an instruction;
    hoist extra waits onto single-wait NoOps just before it (same engine, so
    sequencer order preserves the wait-before-execute semantics)."""
    n_patched = 0
    for f in nc.m.functions:
        for bb in f.blocks:
            new_list = []
            changed = False
            for ins in bb.instructions:
                si = ins.sync_info
                if si is not None and si.on_wait and len(si.on_wait) > max_waits:
                    for k, w in enumerate(si.on_wait):
                        nop = mybir.InstNoOp(name=f"{ins.name}-w{k}", ins=[], outs=[])
                        nop.engine = ins.engine
                        nop.sync_info = mybir.SyncInfo(on_wait=[w], on_update=[])
                        new_list.append(nop)
                    ins.sync_info = mybir.SyncInfo(
                        on_wait=[], on_update=list(si.on_update)
                    )
                    changed = True
                    n_patched += 1
                new_list.append(ins)
            if changed:
                bb.instructions = new_list
    return n_patched


def build_program(debug=(), patch=True, stage=5, for_sim=False, apply_fw=False):
    nc = bass.Bass(num_devices=N_CORES)

    # ---------------- DRAM inputs ----------------
    d_emb = nc.dram_tensor("emb", [256, D], BF16, kind="ExternalInput")
    d_ident = nc.dram_tensor("ident", [P, P], BF16, kind="ExternalInput")
    d_oet = nc.dram_tensor("oet", [256, TL], BF16, kind="ExternalInput")
    d_og = nc.dram_tensor("og", [TL, SEG], BF16, kind="ExternalInput")
    d_otg = nc.dram_tensor("otg", [SEG, TL], BF16, kind="ExternalInput")
    d_icnt = nc.dram_tensor("icnt", [SEG], F32, kind="ExternalInput")
    d_selm = nc.dram_tensor("selm", [P, 2], F32, kind="ExternalInput")
    d_fw = nc.dram_tensor("fw", [D], F32, kind="ExternalInput")
    d_cos2 = nc.dram_tensor("cos2", [P, 2, TL], BF16, kind="ExternalInput")
    d_sin2 = nc.dram_tensor("sin2", [P, 2, TL], BF16, kind="ExternalInput")
    d_wq = nc.dram_tensor("wq", [L, D, D], BF16, kind="ExternalInput")
    d_wqr = nc.dram_tensor("wqr", [L, D, D], BF16, kind="ExternalInput")
    d_wk = nc.dram_tensor("wk", [L, D, D], BF16, kind="ExternalInput")
    d_wkr = nc.dram_tensor("wkr", [L, D, D], BF16, kind="ExternalInput")
    d_wv = nc.dram_tensor("wv", [L, D, D], BF16, kind="ExternalInput")
    d_wo = nc.dram_tensor("wo", [L, D, D], BF16, kind="ExternalInput")
    d_w1 = nc.dram_tensor("w1", [L, D, FF], BF16, kind="ExternalInput")
    d_w2 = nc.dram_tensor("w2", [L, D, FF], BF16, kind="ExternalInput")
    d_w3 = nc.dram_tensor("w3", [L, FF, D], BF16, kind="ExternalInput")

    d_y = nc.dram_tensor("y", [TL, D], F32, kind="ExternalOutput")
    d_ss = nc.dram_tensor("ss", [SEG, D], BF16, kind="ExternalOutput")
    dbg_out = {}

    def dbg(name, shape, dtype=F32):
        if name in debug:
            dbg_out[name] = nc.dram_tensor(
                "dbg_" + name, shape, dtype, kind="ExternalOutput"
            )
            return dbg_out[name]
        return None

    with tile.TileContext(nc) as tc:
        with (
            tc.tile_pool(name="state", bufs=1) as state,
            tc.tile_pool(name="aux", bufs=1) as aux,
            tc.tile_pool(name="wsm", bufs=1) as wsm,
            tc.tile_pool(name="wff", bufs=4) as wff,
            tc.tile_pool(name="stp", bufs=2) as stp,
            tc.tile_pool(name="tmp", bufs=3) as tmp,
            tc.tile_pool(name="psum", bufs=4, space="PSUM") as psum,
            tc.tile_pool(name="psum_st", bufs=2, space="PSUM") as psum_st,
            tc.tile_pool(name="dram", bufs=1, space="DRAM") as dram,
        ):
            # ---- persistent state ----
            x_sb = state.tile([P, NT, D], F32, tag="x")          # residual
            cos2 = state.tile([P, 2, TL], BF16, tag="cos2")
            sin2 = state.tile([P, 2, TL], BF16, tag="sin2")
            ident = state.tile([P, P], BF16, tag="ident")
            selm = state.tile([P, 2], F32, tag="selm")
            eps_sb = state.tile([P, 1], F32, tag="eps")
            nc.vector.memset(eps_sb[:], EPS)

            # h (both halves, transposed): [mc, {0:local,1:remote}, tok]
            hTf = state.tile([P, DC, 2, TL], BF16, tag="hTf")
            # k (rope'd): A = heads 0-3 (mc pairs 0,1), B = heads 4-7
            kTfA = state.tile([P, 2, 2, TL], BF16, tag="kTfA")
            kTfB = state.tile([P, 2, 2, TL], BF16, tag="kTfB")
            # v (+ ones col): [w, kt, h, hd+1]
            vf = state.tile([P, 2, NKT, H, HD + 1], BF16, tag="vf")
            qT = state.tile([P, DC, TL], BF16, tag="qT")
            # local-wave av partial spill (bf16: [h, q-tile, hd+1])
            avp = state.tile([P, H, NT, HD + 1], BF16, tag="avp")

            # ---- embedding: x = onehot @ table (inputs loaded first) ----
            embt = aux.tile([P, 2, D], BF16, tag="embt")
            oet = stp.tile([P, 2, TL], BF16, tag="st")
            nc.sync.dma_start(oet[:], d_oet.rearrange("(c p) t -> p c t", p=P))
            nc.sync.dma_start(embt[:], d_emb.rearrange("(c p) d -> p c d", p=P))
            nc.sync.dma_start(ident[:], d_ident[:])
            nc.sync.dma_start(cos2[:], d_cos2[:])
            nc.sync.dma_start(sin2[:], d_sin2[:])
            nc.sync.dma_start(selm[:], d_selm[:])
            for t in range(NT):
                ps = psum.tile([P, 512], F32, tag="mm")
                for kc in range(2):
                    nc.tensor.matmul(
                        ps[:],
                        oet[:, kc, t * P : (t + 1) * P],
                        embt[:, kc, :],
                        start=(kc == 0),
                        stop=(kc == 1),
                    )
                nc.vector.tensor_copy(x_sb[:, t, :], ps[:])

            def rmsnorm_tile(t, out_tile, out_slice):
                """out = x_sb[:,t,:] * rsqrt(mean(x^2)+eps), bf16."""
                xsq = tmp.tile([P, D], BF16, tag="h")
                ssq = tmp.tile([P, 1], F32, tag="ssq")
                nc.scalar.activation(
                    xsq[:], x_sb[:, t, :], ACT.Square, accum_out=ssq[:]
                )
                nc.scalar.activation(
                    ssq[:], ssq[:], ACT.Sqrt, bias=eps_sb[:], scale=1.0 / D
                )
                nc.vector.reciprocal(ssq[:], ssq[:])
                nc.vector.tensor_scalar_mul(out_tile[out_slice], x_sb[:, t, :], ssq[:])

            def transpose_block(dst_ap, src_ap, use_act=False):
                """[128,128] bf16 transpose via PE; psum->sbuf copy."""
                pt = psum.tile([P, P], BF16, tag="mm", name="pt")
                nc.tensor.transpose(pt[:], src_ap, ident[:])
                if use_act:
                    nc.scalar.copy(dst_ap, pt[:])
                else:
                    nc.vector.tensor_copy(dst_ap, pt[:])

            # pooling index matrices: issue on the idle Act DMA queue so they
            # don't delay weight loads on the SP queue
            og_e = aux.tile([P, NT, SEG], BF16, tag="og")
            otg_e = aux.tile([P, SC, TL], BF16, tag="otg")
            icnt_e = aux.tile([P, SC, 1], F32, tag="icnt")
            nc.scalar.dma_start(og_e[:], d_og.rearrange("(c p) s -> p c s", p=P))
            nc.scalar.dma_start(otg_e[:], d_otg.rearrange("(c p) t -> p c t", p=P))
            nc.scalar.dma_start(
                icnt_e[:], d_icnt.rearrange("(c p) -> p c", p=P)[:, :, None]
            )

            def prologue(l, h_all):
                """rmsnorm + transpose local h into hTf, DMA to bh_in, start
                the pair-AllGather. Returns (bh_out,)."""
                for t in range(NT):
                    rmsnorm_tile(t, h_all, np.s_[:, t, :])
                bh_in = dram.tile([D * TL], BF16, tag="bkv_in")
                bh_out = dram.tile([2 * D * TL], BF16, tag="bkv_out")
                for mc in range(DC):
                    for t in range(NT):
                        transpose_block(
                            hTf[:, mc, 0, t * P : (t + 1) * P],
                            h_all[:, t, mc * P : (mc + 1) * P],
                            use_act=True,
                        )
                    nc.sync.dma_start(
                        bh_in[mc * P * TL : (mc + 1) * P * TL].rearrange(
                            "(p x) -> p x", p=P
                        ),
                        hTf[:, mc, 0, :],
                    )
                nc.gpsimd.collective_compute(
                    "AllGather",
                    ALU.bypass,
                    replica_groups=[[0, 1], [2, 3], [4, 5], [6, 7]],
                    ins=[bh_in[:].opt()],
                    outs=[bh_out[:].opt()],
                )
                return bh_out

            def proj_rope(w_sb, wr_sb, out_tile, out_pre, w, cos_t, sin_t):
                """rope'd projection of hTf[:, :, w, :] -> out_tile.

                out_pre(mc, nh) -> (ap_a, ap_b, out_ap) selects psum/psum
                shapes and destination slices; nh iterates 512-wide chunks of
                TL."""
                for mc in range(DC):
                    for nh in range(TL // 512):
                        nsl = np.s_[nh * 512 : (nh + 1) * 512]
                        pa = psum.tile([P, 512], F32, tag="mm", name="pa")
                        pb = psum.tile([P, 512], F32, tag="mm", name="pb")
                        for kc in range(DC):
                            nc.tensor.matmul(
                                pa[:],
                                w_sb[:, kc, mc * P : (mc + 1) * P],
                                hTf[:, kc, w, nsl],
                                start=(kc == 0),
                                stop=(kc == DC - 1),
                            )
                        for kc in range(DC):
                            nc.tensor.matmul(
                                pb[:],
                                wr_sb[:, kc, mc * P : (mc + 1) * P],
                                hTf[:, kc, w, nsl],
                                start=(kc == 0),
                                stop=(kc == DC - 1),
                            )
                        t1 = tmp.tile([P, 512], BF16, tag="rope")
                        t2 = tmp.tile([P, 512], BF16, tag="rope")
                        nc.vector.tensor_mul(t1[:], pa[:], cos_t[:, nsl])
                        nc.vector.tensor_mul(t2[:], pb[:], sin_t[:, nsl])
                        nc.vector.tensor_add(out_pre(mc, nh), t1[:], t2[:])

            def v_proj(wv_sb, w):
                for t in range(NKT):
                    ps = psum.tile([P, 512], F32, tag="mm")
                    for kc in range(DC):
                        nc.tensor.matmul(
                            ps[:],
                            hTf[:, kc, w, t * P : (t + 1) * P],
                            wv_sb[:, kc, :],
                            start=(kc == 0),
                            stop=(kc == DC - 1),
                        )
                    nc.vector.tensor_copy(
                        vf[:, w, t, :, :HD], ps[:].rearrange("p (h d) -> p h d", h=H)
                    )
                    nc.vector.memset(vf[:, w, t, :, HD], 1.0)

            def emit_scores(h, qb, w):
                """scores + exp for block (head h, q-block qb) vs wave-w keys.
                Returns st tile [P, NKT, QBW]."""
                po = (h % 2) * HD
                hc = h // 2
                ksrc = kTfA if hc < 2 else kTfB
                qsl = np.s_[qb * QBW : (qb + 1) * QBW]
                st_sb = stp.tile([P, 2, NKT // 2, QBW], BF16, tag="st", name="st_sb")
                for k4 in range(NKT // 4):
                    pst = psum_st.tile([P, 4, QBW], F32, tag="st")
                    for j in range(4):
                        kt = k4 * 4 + j
                        nc.tensor.matmul(
                            pst[:, j, :],
                            ksrc[po : po + HD, hc % 2, w, kt * P : (kt + 1) * P],
                            qT[po : po + HD, hc, qsl],
                            start=True,
                            stop=True,
                        )
                    nc.scalar.activation(
                        st_sb.rearrange("p a b q -> p (a b) q")[
                            :, k4 * 4 : (k4 + 1) * 4, :
                        ],
                        pst[:],
                        ACT.Exp,
                        scale=1.0 / math.sqrt(HD),
                    )
                return st_sb.rearrange("p a b q -> p (a b) q")

            def emit_av_local(h, qb, st):
                for qt in range(QBW // P):
                    tix = qb * (QBW // P) + qt
                    pav = psum.tile([P, 512], F32, tag="mm", name="pav")
                    for kt in range(NKT):
                        nc.tensor.matmul(
                            pav[:, : HD + 1],
                            st[:, kt, qt * P : (qt + 1) * P],
                            vf[:, 0, kt, h, :],
                            start=(kt == 0),
                            stop=(kt == NKT - 1),
                        )
                    nc.vector.tensor_copy(avp[:, h, tix, :], pav[:, : HD + 1])

            def emit_av_remote(h, qb, st, o_sb):
                for qt in range(QBW // P):
                    tix = qb * (QBW // P) + qt
                    pav = psum.tile([P, 512], F32, tag="mm", name="pav")
                    for kt in range(NKT):
                        nc.tensor.matmul(
                            pav[:, : HD + 1],
                            st[:, kt, qt * P : (qt + 1) * P],
                            vf[:, 1, kt, h, :],
                            start=(kt == 0),
                            stop=(kt == NKT - 1),
                        )
                    avf = tmp.tile([P, HD + 1], F32, tag="avf")
                    nc.vector.tensor_add(avf[:], pav[:, : HD + 1], avp[:, h, tix, :])
                    rcp = tmp.tile([P, 1], F32, tag="rcp")
                    nc.vector.reciprocal(rcp[:], avf[:, HD : HD + 1])
                    nc.vector.tensor_scalar_mul(
                        o_sb[:, tix, h * HD : (h + 1) * HD], avf[:, :HD], rcp[:]
                    )

            # ================= layers =================
            h_all = stp.tile([P, NT, D], BF16, tag="o_or_h", bufs=1)
            o_sb = h_all  # same storage: h_all consumed before o is written
            for l in range(L if stage >= 2 else 1):
                # weight DMAs issued before the prologue's bh_in DMAs so the
                # in-order SP queue doesn't delay them behind the transposes
                w_sb = wsm.tile([P, DC, D], BF16, tag="wq")
                wr_sb = wsm.tile([P, DC, D], BF16, tag="wqr")
                wk_sb = wsm.tile([P, DC, D], BF16, tag="wk")
                wkr_sb = wsm.tile([P, DC, D], BF16, tag="wkr")
                wv_sb = wsm.tile([P, DC, D], BF16, tag="wv")
                nc.sync.dma_start(w_sb[:], d_wq[l].rearrange("(c p) n -> p c n", p=P))
                nc.sync.dma_start(wr_sb[:], d_wqr[l].rearrange("(c p) n -> p c n", p=P))
                nc.sync.dma_start(wk_sb[:], d_wk[l].rearrange("(c p) n -> p c n", p=P))
                nc.sync.dma_start(wkr_sb[:], d_wkr[l].rearrange("(c p) n -> p c n", p=P))
                nc.sync.dma_start(wv_sb[:], d_wv[l].rearrange("(c p) n -> p c n", p=P))

                bh_out = prologue(l, h_all)

                # ---- q projection + rope from local h (overlaps collective)
                proj_rope(
                    w_sb, wr_sb, qT, lambda mc, nh: qT[:, mc, nh * 512 : (nh + 1) * 512],
                    0, cos2[:, 0, :], sin2[:, 0, :],
                )

                def k_out(w):
                    def _f(mc, nh):
                        ktile = kTfA if mc < 2 else kTfB
                        return ktile[:, mc % 2, w, nh * 512 : (nh + 1) * 512]
                    return _f

                proj_rope(wk_sb, wkr_sb, None, k_out(0), 0,
                          cos2[:, 0, :], sin2[:, 0, :])

                # ---- v local ----
                v_proj(wv_sb, 0)

                if stage < 3:
                    break

                def emit_select():
                    # select partner half h into hTf[:, :, 1, :]; emitted
                    # late in the local wave so the collective has landed and
                    # the DVE ops run in the wave's shadow
                    for mc in range(DC):
                        hst = stp.tile([P, 2, TL], BF16, tag="st", name="hst")
                        for r in range(2):
                            nc.sync.dma_start(
                                hst[:, r, :],
                                bh_out[r * D * TL + mc * P * TL :][
                                    : P * TL
                                ].rearrange("(p x) -> p x", p=P),
                            )
                        hsel = tmp.tile([P, TL], BF16, tag="hsel", bufs=1)
                        nc.vector.tensor_scalar_mul(
                            hsel[:], hst[:, 0, :], selm[:, 0:1]
                        )
                        nc.vector.scalar_tensor_tensor(
                            hTf[:, mc, 1, :], hst[:, 1, :], selm[:, 1:2], hsel[:],
                            op0=ALU.mult, op1=ALU.add,
                        )

                # ---- local attention wave (overlaps collective) ----
                prev = None
                for h in range(H):
                    for qb in range(NQB):
                        st = emit_scores(h, qb, 0)
                        if prev is not None:
                            emit_av_local(*prev)
                        prev = (h, qb, st)
                    if h == H - 2:
                        # flush the pending av before hst reuses the st ring:
                        # a slot's reader must be emitted before another tile
                        # claims it, or the hst DMA can overwrite live scores
                        emit_av_local(*prev)
                        prev = None
                        emit_select()
                if prev is not None:
                    emit_av_local(*prev)

                # ---- k/v remote ----
                proj_rope(wk_sb, wkr_sb, None, k_out(1), 1,
                          cos2[:, 1, :], sin2[:, 1, :])
                v_proj(wv_sb, 1)

                # ---- remote attention wave, qb-major; wo + FFN pipelined ----
                wo_sb = wsm.tile([P, DC, D], BF16, tag="wo")
                nc.sync.dma_start(wo_sb[:], d_wo[l].rearrange("(c p) n -> p c n", p=P))
                w3_sb = wsm.tile([P, FFC, D], BF16, tag="w3")
                nc.sync.dma_start(w3_sb[:], d_w3[l].rearrange("(c p) n -> p c n", p=P))

                oT = state.tile([P, DC, 2, P], BF16, tag="oT")  # per-qb transposed o
                hT2 = state.tile([P, DC, 512], BF16, tag="hT2")  # per-th ffn input

                def ffn_half(th):
                    # rmsnorm + transpose the 4 fresh x tiles
                    for t in range(th * 4, th * 4 + 4):
                        h2 = tmp.tile([P, D], BF16, tag="h")
                        rmsnorm_tile(t, h2, np.s_[:])
                        for mc in range(DC):
                            transpose_block(
                                hT2[:, mc, (t - th * 4) * P : (t - th * 4 + 1) * P],
                                h2[:, mc * P : (mc + 1) * P],
                            )
                    h12 = state.tile([P, FFC, 512], BF16, tag="h12")
                    for mc2 in range(FFC // 2):
                        msl = np.s_[:, mc2 * 2 * P : (mc2 + 1) * 2 * P]
                        w1c = wff.tile([P, DC, 2 * P], BF16, tag="wffc")
                        w2c = wff.tile([P, DC, 2 * P], BF16, tag="wffc")
                        nc.sync.dma_start(
                            w1c[:], d_w1[l][msl].rearrange("(c p) n -> p c n", p=P)
                        )
                        nc.sync.dma_start(
                            w2c[:], d_w2[l][msl].rearrange("(c p) n -> p c n", p=P)
                        )
                        for mi in range(2):
                            mc = mc2 * 2 + mi
                            p1 = psum.tile([P, 512], F32, tag="mm", name="p1")
                            p2 = psum.tile([P, 512], F32, tag="mm", name="p2")
                            for kc in range(DC):
                                nc.tensor.matmul(
                                    p1[:],
                                    w1c[:, kc, mi * P : (mi + 1) * P],
                                    hT2[:, kc, :],
                                    start=(kc == 0),
                                    stop=(kc == DC - 1),
                                )
                            for kc in range(DC):
                                nc.tensor.matmul(
                                    p2[:],
                                    w2c[:, kc, mi * P : (mi + 1) * P],
                                    hT2[:, kc, :],
                                    start=(kc == 0),
                                    stop=(kc == DC - 1),
                                )
                            sl = tmp.tile([P, 512], BF16, tag="h")
                            if for_sim:
                                nc.scalar.activation(sl[:], p1[:], ACT.Sigmoid)
                                u = tmp.tile([P, 512], BF16, tag="h")
                                nc.vector.tensor_mul(u[:], p1[:], sl[:])
                                nc.vector.tensor_mul(h12[:, mc, :], p2[:], u[:])
                            else:
                                nc.scalar.activation(sl[:], p1[:], ACT.Silu)
                                nc.vector.tensor_mul(h12[:, mc, :], p2[:], sl[:])
                    for t in range(th * 4, th * 4 + 4):
                        ps = psum.tile([P, 512], F32, tag="mm")
                        for kc in range(FFC):
                            nc.tensor.matmul(
                                ps[:],
                                h12[:, kc, (t - th * 4) * P : (t - th * 4 + 1) * P],
                                w3_sb[:, kc, :],
                                start=(kc == 0),
                                stop=(kc == FFC - 1),
                            )
                        nc.vector.tensor_add(x_sb[:, t, :], x_sb[:, t, :], ps[:])

                prev = None
                for qb in range(NQB):
                    for h in range(H):
                        st = emit_scores(h, qb, 1)
                        if prev is not None:
                            emit_av_remote(*prev, o_sb)
                        prev = (h, qb, st)
                    emit_av_remote(*prev, o_sb)
                    prev = None
                    # o[qb] complete -> transpose + wo + residual
                    for qt in range(QBW // P):
                        tix = qb * (QBW // P) + qt
                        for mc in range(DC):
                            transpose_block(
                                oT[:, mc, qt, :],
                                o_sb[:, tix, mc * P : (mc + 1) * P],
                            )
                        ps = psum.tile([P, 512], F32, tag="mm")
                        for mc in range(DC):
                            nc.tensor.matmul(
                                ps[:],
                                oT[:, mc, qt, :],
                                wo_sb[:, mc, :],
                                start=(mc == 0),
                                stop=(mc == DC - 1),
                            )
                        nc.vector.tensor_add(x_sb[:, tix, :], x_sb[:, tix, :], ps[:])
                    if stage >= 4 and qb % 2 == 1:
                        ffn_half(qb // 2)

            # ================= final norm + segment pooling =================
            xn = o_sb  # reuse storage
            for t in range(NT):
                rmsnorm_tile(t, xn, np.s_[:, t, :])

            if stage < 5:
                for t in range(NT):
                    xf = tmp.tile([P, D], F32, tag="f32t", bufs=2)
                    nc.vector.tensor_copy(xf[:], xn[:, t, :])
                    nc.sync.dma_start(
                        d_y.rearrange("(t p) d -> t p d", p=P)[t], xf[:]
                    )
                zss = tmp.tile([P, SC, D], BF16, tag="segsum", bufs=1)
                nc.vector.memset(zss[:], 0.0)
                nc.sync.dma_start(d_ss.rearrange("(c p) d -> p c d", p=P), zss[:])
            if stage >= 5:
                if apply_fw:
                    fw_bc = aux.tile([P, D], F32, tag="fw_bc")
                    nc.sync.dma_start(
                        fw_bc[:],
                        bass.AP(tensor=d_fw, offset=0, ap=[[0, P], [1, D]]),
                    )

                segsum_bf = aux.tile([P, SC, D], BF16, tag="segsum_bf")
                for mc in range(SC):
                    ps = psum.tile([P, 512], F32, tag="mm")
                    for kt in range(NT):
                        nc.tensor.matmul(
                            ps[:],
                            og_e[:, kt, mc * P : (mc + 1) * P],
                            xn[:, kt, :],
                            start=(kt == 0),
                            stop=(kt == NT - 1),
                        )
                    nc.scalar.copy(segsum_bf[:, mc, :], ps[:])
                # per-segment sums out for the host-side boundary fix
                nc.sync.dma_start(
                    d_ss.rearrange("(c p) d -> p c d", p=P), segsum_bf[:]
                )

                # seg_mean = partial * icnt (icnt = 0.5/cnt_total); the
                # cross-half contribution for the one boundary segment is
                # added on the host from the ss outputs.
                segmean = aux.tile([P, SC, D], BF16, tag="segmean")
                for mc in range(SC):
                    nc.vector.tensor_scalar_mul(
                        segmean[:, mc, :], segsum_bf[:, mc, :], icnt_e[:, mc, :]
                    )

                # out = 0.5*xn + scatter(segmean)
                for t in range(NT):
                    ps = psum.tile([P, 512], F32, tag="mm")
                    for kc in range(SC):
                        nc.tensor.matmul(
                            ps[:],
                            otg_e[:, kc, t * P : (t + 1) * P],
                            segmean[:, kc, :],
                            start=(kc == 0),
                            stop=(kc == SC - 1),
                        )
                    out_t = tmp.tile([P, D], F32, tag="f32t", bufs=2)
                    nc.vector.scalar_tensor_tensor(
                        out_t[:], xn[:, t, :], ALPHA, ps[:],
                        op0=ALU.mult, op1=ALU.add,
                    )
                    if apply_fw:
                        nc.vector.tensor_mul(out_t[:], out_t[:], fw_bc[:])
                    nc.sync.dma_start(
                        d_y.rearrange("(t p) d -> t p d", p=P)[t], out_t[:]
                    )

            # ---- debug taps ----
            if "x0" in debug:
                nc.sync.dma_start(
                    dbg_out["x0"].rearrange("(t p) d -> t p d", p=P)[:], x_sb[:]
                )

    if patch:
        split_multiwait_drains(nc)
    return nc


# ----------------------------------------------------------------------------
# host side
# ----------------------------------------------------------------------------


def _rot_cols(w):
    """Columns permuted/negated so (h @ w_rot) == rotate_half(h @ w)."""
    wr = np.empty_like(w)
    for hb in range(0, D, HD):
        wr[:, hb : hb + HD // 2] = -w[:, hb + HD // 2 : hb + HD]
        wr[:, hb + HD // 2 : hb + HD] = w[:, hb : hb + HD // 2]
    return wr


def _to_bf16(a):
    return np.asarray(a, dtype=np.float32).astype(ml_dtypes.bfloat16)


def host_prep(inputs):
    tokens = np.clip(np.asarray(inputs["tokens"]), 0, 255).astype(np.int64)
    emb = np.asarray(inputs["embed_table"], np.float32)
    attn_w = np.asarray(inputs["attn_norm_w"], np.float32)
    ffn_w = np.asarray(inputs["ffn_norm_w"], np.float32)
    fin_w = np.asarray(inputs["final_norm_w"], np.float32)
    wq = np.asarray(inputs["wq"], np.float32) * attn_w[:, :, None]
    wk = np.asarray(inputs["wk"], np.float32) * attn_w[:, :, None]
    wv = np.asarray(inputs["wv"], np.float32) * attn_w[:, :, None]
    wo = np.asarray(inputs["wo"], np.float32)
    w1 = np.asarray(inputs["w1"], np.float32) * ffn_w[:, :, None]
    w2 = np.asarray(inputs["w2"], np.float32) * ffn_w[:, :, None]
    w3 = np.asarray(inputs["w3"], np.float32)

    wqr = np.stack([_rot_cols(wq[l]) for l in range(L)])
    wkr = np.stack([_rot_cols(wk[l]) for l in range(L)])

    shared = {
        "fw": fin_w,
        "ident": _to_bf16(np.eye(P, dtype=np.float32)),
        "emb": _to_bf16(emb),
        "wq": _to_bf16(wq),
        "wqr": _to_bf16(wqr),
        "wk": _to_bf16(wk),
        "wkr": _to_bf16(wkr),
        "wv": _to_bf16(wv),
        "wo": _to_bf16(wo),
        "w1": _to_bf16(w1),
        "w2": _to_bf16(w2),
        "w3": _to_bf16(w3),
    }

    # rope tables (rows: 2 head-blocks of 64; same pattern for every head pair)
    inv = 1.0 / (10000.0 ** (np.arange(0, HD, 2, dtype=np.float64) / HD))  # (32,)
    in_maps = []
    for c in range(N_CORES):
        b, half = c // 2, c % 2
        tok = tokens[b, half * TL : (half + 1) * TL]
        posf = np.arange(T, dtype=np.float64)
        ff = posf[None, :] * inv[:, None]  # (32, T)
        cos32, sin32 = np.cos(ff), np.sin(ff)
        cos64 = np.concatenate([cos32, cos32], 0)  # (64, T)
        sin64 = np.concatenate([sin32, sin32], 0)
        cosF = np.concatenate([cos64, cos64], 0)  # (128, T)
        sinF = np.concatenate([sin64, sin64], 0)
        loc_sl = np.s_[:, half * TL : (half + 1) * TL]
        rem_sl = np.s_[:, (1 - half) * TL : (2 - half) * TL]
        cos2 = np.stack([cosF[loc_sl], cosF[rem_sl]], axis=1)  # (128, 2, TL)
        sin2 = np.stack([sinF[loc_sl], sinF[rem_sl]], axis=1)

        selm = np.zeros((P, 2), np.float32)
        selm[:, 1 - half] = 1.0  # partner occupies row (1-half) of bh_out

        oet = np.zeros((256, TL), np.float32)
        oet[tok, np.arange(TL)] = 1.0

        # segments
        is_sep = SEP_TABLE[tokens[b]]
        seg = np.cumsum(is_sep.astype(np.int64))  # inclusive, full row
        cnt = np.bincount(seg, minlength=seg[-1] + 1).astype(np.float64)
        ids = seg[half * TL : (half + 1) * TL]
        base = ids[0]
        loc = ids - base
        S_loc = int(loc[-1]) + 1
        assert S_loc <= SEG, f"too many segments {S_loc}"
        og = np.zeros((TL, SEG), np.float32)
        og[np.arange(TL), loc] = 1.0
        icnt = np.ones(SEG, np.float64)
        icnt[:S_loc] = 0.5 / np.maximum(cnt[base : base + S_loc], 1.0)

        in_maps.append(
            dict(
                shared,
                oet=_to_bf16(oet),
                og=_to_bf16(og),
                otg=_to_bf16(og.T.copy()),
                icnt=icnt.astype(np.float32),
                selm=selm,
                cos2=_to_bf16(cos2),
                sin2=_to_bf16(sin2),
            )
        )
    return in_maps


class Runner:
    """Compile once; keep inputs device-resident; re-upload only changed data."""

    def __init__(self, nc):
        import jax
        import jax.numpy as jnp
        from jax.experimental.shard_map import shard_map
        from jax.sharding import Mesh, PartitionSpec
        import concourse.mybir as mybir_
        from concourse import bass2jax

        bass2jax.install_neuronx_cc_hook()
        self.jax = jax
        self.nc = nc
        in_names, out_names, out_avals, zero_outs = [], [], [], []
        for alloc in nc.m.functions[0].allocations:
            if not isinstance(mybir_.MemoryLocationSet, type) or not isinstance(
                alloc, mybir_.MemoryLocationSet
            ):
                continue
            name = alloc.memorylocations[0].name
            if alloc.kind == "ExternalInput":
                if nc.partition_id_tensor is None or name != nc.partition_id_tensor.name:
                    in_names.append(name)
            elif alloc.kind == "ExternalOutput":
                shape = tuple(alloc.tensor_shape)
                dtype = mybir_.dt.np(alloc.dtype)
                out_names.append(name)
                out_avals.append(jax.core.ShapedArray(shape, dtype))
                zero_outs.append(np.zeros(shape, dtype))
        self.n_params = len(in_names)
        self.in_names = list(in_names)
        self.out_names = out_names
        all_in_names = in_names + out_names
        partition_name = nc.partition_id_tensor.name if nc.partition_id_tensor else None
        if partition_name is not None:
            all_in_names = all_in_names + [partition_name]

        def _body(*args):
            operands = list(args)
            if partition_name is not None:
                operands.append(bass2jax.partition_id_tensor())
            outs = bass2jax._bass_exec_p.bind(
                *operands,
                out_avals=tuple(out_avals),
                in_names=tuple(all_in_names),
                out_names=tuple(out_names),
                lowering_input_output_aliases=(),
                sim_require_finite=True,
                sim_require_nnan=True,
                nc=nc,
            )
            return tuple(outs)

        devices = jax.devices()[:N_CORES]
        mesh = Mesh(np.asarray(devices), ("core",))
        n_in = self.n_params + len(out_names)
        self.sharded = jax.jit(
            shard_map(
                _body,
                mesh=mesh,
                in_specs=(PartitionSpec("core"),) * n_in,
                out_specs=(PartitionSpec("core"),) * len(out_names),
                check_rep=False,
            ),
            keep_unused=True,
        )
        self.mesh = mesh
        self.zero_outs = zero_outs
        self._dev_zero = None
        self._cache_np = {}
        self._cache_dev = {}

    def _put(self, name, arrs):
        """Concat per-core numpy arrays and put sharded on device (cached)."""
        import jax
        from jax.sharding import NamedSharding, PartitionSpec

        cached = self._cache_np.get(name)
        if cached is not None and all(
            a is b or (a.shape == b.shape and np.array_equal(a, b))
            for a, b in zip(cached, arrs)
        ):
            return self._cache_dev[name]
        glob = np.concatenate([np.asarray(a) for a in arrs], axis=0)
        dev = jax.device_put(glob, NamedSharding(self.mesh, PartitionSpec("core")))
        self._cache_np[name] = [np.asarray(a) for a in arrs]
        self._cache_dev[name] = dev
        return dev

    def __call__(self, in_maps):
        import jax
        from jax.sharding import NamedSharding, PartitionSpec

        args = [
            self._put(name, [m[name] for m in in_maps]) for name in self.in_names
        ]
        if self._dev_zero is None:
            self._dev_zero = [
                jax.device_put(
                    np.zeros((N_CORES * z.shape[0], *z.shape[1:]), z.dtype),
                    NamedSharding(self.mesh, PartitionSpec("core")),
                )
                for z in self.zero_outs
            ]
        outs = self.sharded(*args, *self._dev_zero)
        outs = [np.asarray(o) for o in outs]
        return {
            name: outs[i].reshape(N_CORES, *self.zero_outs[i].shape)
            for i, name in enumerate(self.out_names)
        }


_RUNNER = None
_RUNNER_FLAGS = None


def _get_runner(apply_fw=False):
    global _RUNNER, _RUNNER_FLAGS
    if _RUNNER is None or _RUNNER_FLAGS != (apply_fw,):
        nc = build_program(apply_fw=apply_fw)
        _RUNNER = Runner(nc)
        _RUNNER_FLAGS = (apply_fw,)
    return _RUNNER


def kernel(**inputs):
    apply_fw = not np.allclose(np.asarray(inputs["final_norm_w"]), 1.0)
    runner = _get_runner(apply_fw=apply_fw)
    in_maps = host_prep(inputs)
    res = runner(in_maps)
    y = res["y"]
    ss = res["ss"]
    out = np.zeros((B, T, D), np.float32)
    for c in range(N_CORES):
        b, half = c // 2, c % 2
        out[b, half * TL : (half + 1) * TL, :] = y[c]

    # host-side cross-half boundary-segment correction
    tokens = np.clip(np.asarray(inputs["tokens"]), 0, 255).astype(np.int64)
    fin_w = np.asarray(inputs["final_norm_w"], np.float32)
    for b in range(B):
        is_sep = SEP_TABLE[tokens[b]]
        seg = np.cumsum(is_sep.astype(np.int64))
        if seg[TL - 1] != seg[TL]:
            continue  # separator right at the half boundary: no spanning seg
        s_b = int(seg[TL])
        cnt = np.bincount(seg, minlength=s_b + 1).astype(np.float64)
        corr = np.float32(0.5 / max(cnt[s_b], 1.0))
        cA, cB = 2 * b, 2 * b + 1
        baseA = int(seg[0])
        bpA = ss[cA][s_b - baseA].astype(np.float32)
        bpB = ss[cB][0].astype(np.float32)  # boundary seg is B's first segment
        if apply_fw:
            bpA = bpA * fin_w
            bpB = bpB * fin_w
        maskA = seg[:TL] == s_b
        maskB = seg[TL:] == s_b
        out[b, :TL][maskA] += corr * bpB[None, :]
        out[b, TL:][maskB] += corr * bpA[None, :]
    return out
